# revision 1
# baseline (speedup 1.0000x reference)
"""nn_Block dense_transformer kernel for 8 TRN2 NeuronCores.

Self-contained: builds the Bass/Tile program, shards inputs across 8 cores
(sequence-parallel, 4 cores per batch element), runs via a cached jitted
shard_map over jax's neuron devices, gathers the full output.
"""
import os
import numpy as np
import ml_dtypes


from contextlib import ExitStack
from dataclasses import dataclass

import concourse.bass as bass
import concourse.mybir as mybir
import concourse.tile as tile

F32 = mybir.dt.float32
F32R = mybir.dt.float32r
BF16 = mybir.dt.bfloat16
P = 128
ALU = mybir.AluOpType
ACTF = mybir.ActivationFunctionType


@dataclass
class Cfg:
    dim: int = 1024
    heads: int = 16
    hd: int = 64
    hidden: int = 4096
    T: int = 512          # tokens per core
    group: int = 4        # cores per batch group
    n_cores: int = 8
    eps: float = 1e-5
    # build-time specialization flags (host inspects actual input values)
    apply_ln1_gb: bool = False
    apply_ln2_gb: bool = False
    apply_qkv_bias: bool = False
    apply_proj_bias: bool = False
    apply_fc1_bias: bool = False
    apply_fc2_bias: bool = False
    fake_ag: bool = False  # timing-only: skip collective, read own kv as all ranks
    exp_batch: int = 1    # j-steps batched per exp activation (PSUM: 2*eb banks)
    stop_after: str = "full"  # bisect: ln1|qkv|ag|kvload|attn|proj|ln2|fc1|full
    attn_parts: str = "full"  # bisect: qk|qkexp|av|full

    @property
    def dch(self):
        return self.dim // P

    @property
    def hch(self):
        return self.hidden // P

    @property
    def kt(self):
        return (self.group * self.T) // P

    @property
    def tpr(self):
        return self.T // P

    @property
    def pairs(self):
        return self.heads // 2

    @property
    def vf(self):  # free-dim chunk for the v matmul
        return min(512, self.dim)

    @property
    def scale(self):
        return self.hd ** -0.5


def build_block(nc: bass.Bass, cfg: Cfg):
    dch, hch, kt, T, hd = cfg.dch, cfg.hch, cfg.kt, cfg.T, cfg.hd

    xT = nc.dram_tensor("xT", [cfg.dim, T], F32, kind="ExternalInput").ap()
    wqkv = nc.dram_tensor("wqkv", [cfg.dim, 3 * cfg.dim], BF16, kind="ExternalInput").ap()
    wproj = nc.dram_tensor("wproj", [cfg.dim, cfg.dim], BF16, kind="ExternalInput").ap()
    wfc1 = nc.dram_tensor("wfc1", [cfg.dim, cfg.hidden], BF16, kind="ExternalInput").ap()
    wfc2 = nc.dram_tensor("wfc2", [cfg.hidden, cfg.dim], BF16, kind="ExternalInput").ap()
    outT = nc.dram_tensor("outT", [cfg.dim, T], F32, kind="ExternalOutput").ap()

    ln1_g = ln1_b = ln2_g = ln2_b = None
    if cfg.apply_ln1_gb:
        ln1_g = nc.dram_tensor("ln1_g", [cfg.dim], F32, kind="ExternalInput").ap()
        ln1_b = nc.dram_tensor("ln1_b", [cfg.dim], F32, kind="ExternalInput").ap()
    if cfg.apply_ln2_gb:
        ln2_g = nc.dram_tensor("ln2_g", [cfg.dim], F32, kind="ExternalInput").ap()
        ln2_b = nc.dram_tensor("ln2_b", [cfg.dim], F32, kind="ExternalInput").ap()
    qkv_b = proj_b = fc1_b = fc2_b = None
    if cfg.apply_qkv_bias:
        qkv_b = nc.dram_tensor("qkv_b", [3 * cfg.dim], F32, kind="ExternalInput").ap()
    if cfg.apply_proj_bias:
        proj_b = nc.dram_tensor("proj_b", [cfg.dim], F32, kind="ExternalInput").ap()
    if cfg.apply_fc1_bias:
        fc1_b = nc.dram_tensor("fc1_b", [cfg.hidden], F32, kind="ExternalInput").ap()
    if cfg.apply_fc2_bias:
        fc2_b = nc.dram_tensor("fc2_b", [cfg.dim], F32, kind="ExternalInput").ap()

    # collective bounce buffers (k first so its AllGather can complete and
    # feed QK while v's AllGather still runs)
    half = cfg.dim * T
    cck_in = nc.dram_tensor("cck_in", [half], BF16, kind="Internal").ap()
    cck_out = nc.dram_tensor("cck_out", [cfg.group * half], BF16, kind="Internal").ap()
    ccv_in = nc.dram_tensor("ccv_in", [half], BF16, kind="Internal").ap()
    ccv_out = nc.dram_tensor("ccv_out", [cfg.group * half], BF16, kind="Internal").ap()

    def dram_view(ap, off, shape_strides):
        return bass.AP(tensor=ap.tensor, offset=ap.offset + off,
                       ap=[[s, n] for s, n in shape_strides])

    with tile.TileContext(nc) as tc, ExitStack() as ctx:
        const = ctx.enter_context(tc.tile_pool(name="const", bufs=1))
        rows = ctx.enter_context(tc.tile_pool(name="rows", bufs=2))
        bcast = ctx.enter_context(tc.tile_pool(name="bcast", bufs=2))
        evict = ctx.enter_context(tc.tile_pool(name="evict", bufs=3))
        tmp = ctx.enter_context(tc.tile_pool(name="tmp", bufs=2))
        # pool release is strict LIFO: open long-lived pools first
        x2p = ctx.enter_context(tc.tile_pool(name="x2p", bufs=1))
        es_proj = ExitStack()   # pools freed after proj: xT, oT, wproj
        es_attn = ExitStack()   # pools freed after attention: qT
        xpool = es_proj.enter_context(tc.tile_pool(name="xpool", bufs=1))
        opool = es_proj.enter_context(tc.tile_pool(name="opool", bufs=1))
        wprojp = es_proj.enter_context(tc.tile_pool(name="wproj", bufs=dch))
        qpool = es_attn.enter_context(tc.tile_pool(name="qpool", bufs=1))

        ones_col = const.tile([P, 1], BF16, tag="ones", bufs=1)
        nc.vector.memset(ones_col, 1.0)
        eps_t = const.tile([1, 1], F32, tag="eps", bufs=1)
        nc.vector.memset(eps_t, cfg.eps)

        xT_sb = xpool.tile([P, dch, T], F32, name="xT_sb", tag="xT", bufs=1)
        for c in range(dch):
            nc.sync.dma_start(out=xT_sb[:, c, :], in_=xT[c * P : (c + 1) * P, :])

        def load_cols(vec, nch, nm):
            t = const.tile([P, nch], F32, name=nm, tag=nm, bufs=1)
            nc.sync.dma_start(out=t, in_=vec.rearrange("(c p) -> p c", p=P))
            return t

        ln1g_sb = load_cols(ln1_g, dch, "ln1g") if ln1_g is not None else None
        ln1b_sb = load_cols(ln1_b, dch, "ln1b") if ln1_b is not None else None
        ln2g_sb = load_cols(ln2_g, dch, "ln2g") if ln2_g is not None else None
        ln2b_sb = load_cols(ln2_b, dch, "ln2b") if ln2_b is not None else None
        qkvb_sb = load_cols(qkv_b, 3 * dch, "qkvb") if qkv_b is not None else None
        projb_sb = load_cols(proj_b, dch, "projb") if proj_b is not None else None
        fc1b_sb = load_cols(fc1_b, hch, "fc1b") if fc1_b is not None else None
        fc2b_sb = load_cols(fc2_b, dch, "fc2b") if fc2_b is not None else None

        # ------------- LayerNorm over features (feature-major data) -------------
        def layer_norm(psln, x_sb, out_bf, g_sb, b_sb):
            sum_x = psln.tile([1, T], F32, name="sum_x", tag="sumx", bufs=1)
            sum_sq = psln.tile([1, T], F32, name="sum_sq", tag="sumsq", bufs=1)
            for c in range(dch):
                xb = tmp.tile([P, T], BF16, name="x_bf", tag="xbf")
                nc.vector.tensor_copy(out=xb, in_=x_sb[:, c, :])
                xsq = tmp.tile([P, T], BF16, name="xsq", tag="xsq")
                nc.vector.tensor_mul(xsq, x_sb[:, c, :], x_sb[:, c, :])
                nc.tensor.matmul(sum_x, lhsT=ones_col, rhs=xb,
                                 start=(c == 0), stop=(c == dch - 1))
                nc.tensor.matmul(sum_sq, lhsT=ones_col, rhs=xsq,
                                 start=(c == 0), stop=(c == dch - 1))
            inv_d = 1.0 / cfg.dim
            m_row = rows.tile([1, T], F32, name="m_row")
            var_row = rows.tile([1, T], F32, name="var_row")
            nc.scalar.mul(m_row, sum_x, inv_d)
            nc.vector.tensor_mul(var_row, m_row, m_row)           # m^2
            nc.vector.scalar_tensor_tensor(                        # sum_sq/D - m^2
                out=var_row, in0=sum_sq, scalar=inv_d, in1=var_row,
                op0=ALU.mult, op1=ALU.subtract)
            lnv = rows.tile([1, T], F32, name="lnv")
            rstd = rows.tile([1, T], F32, name="rstd")
            nc.scalar.activation(lnv, var_row, ACTF.Ln, bias=eps_t)
            nc.scalar.activation(rstd, lnv, ACTF.Exp, scale=-0.5)
            r2 = rows.tile([1, T], F32, name="r2")                 # -m * rstd
            nc.vector.scalar_tensor_tensor(
                out=r2, in0=m_row, scalar=-1.0, in1=rstd,
                op0=ALU.mult, op1=ALU.mult)
            r1b = bcast.tile([P, T], F32, name="r1b", bufs=1)
            r2b = bcast.tile([P, T], F32, name="r2b", bufs=1)
            nc.gpsimd.partition_broadcast(r1b, rstd)
            nc.gpsimd.partition_broadcast(r2b, r2)
            for c in range(dch):
                t0 = tmp.tile([P, T], F32, name="ln_t0", tag="t0")
                nc.vector.tensor_mul(t0, x_sb[:, c, :], r1b)
                if g_sb is not None:
                    t1 = tmp.tile([P, T], F32, name="ln_t1", tag="t1")
                    nc.vector.tensor_add(t1, t0, r2b)
                    nc.vector.tensor_scalar(
                        out=out_bf[:, c, :], in0=t1,
                        scalar1=g_sb[:, c : c + 1], scalar2=b_sb[:, c : c + 1],
                        op0=ALU.mult, op1=ALU.add)
                else:
                    nc.vector.tensor_add(out_bf[:, c, :], t0, r2b)

        # ---------------- Phase 1: LN1 + QKV + AllGather ----------------
        qT_sb = qpool.tile([P, dch, T], BF16, name="qT_sb")
        with tc.tile_pool(name="h1", bufs=1) as h1p, \
             tc.tile_pool(name="wqkv", bufs=dch) as wqkvp, \
             tc.tile_pool(name="psln1", bufs=1, space="PSUM") as psln1, \
             tc.tile_pool(name="psqkv", bufs=4, space="PSUM") as psq:
            h1_sb = h1p.tile([P, dch, T], BF16, name="h1_sb")
            layer_norm(psln1, xT_sb, h1_sb, ln1g_sb, ln1b_sb)

            wq = []
            for c in range(dch):
                w = wqkvp.tile([P, 3 * cfg.dim], BF16, name="wqkv_t")
                nc.sync.dma_start(out=w, in_=wqkv[c * P : (c + 1) * P, :])
                wq.append(w)

            def qkv_evict(ps, dst, col_idx):
                if qkvb_sb is not None:
                    nc.vector.tensor_scalar_add(
                        out=dst, in0=ps, scalar1=qkvb_sb[:, col_idx : col_idx + 1])
                else:
                    nc.vector.tensor_copy(out=dst, in_=ps)

            # k and v first: they feed the AllGather, so it can start while
            # the (local-only) q matmuls still run
            for m in range(dch):  # k, feature-major
                ps = psq.tile([P, T], F32, name="ps_qk", tag="ps_qkv")
                for c in range(dch):
                    nc.tensor.matmul(ps, lhsT=wq[c][:, (dch + m) * P : (dch + m + 1) * P],
                                     rhs=h1_sb[:, c, :],
                                     start=(c == 0), stop=(c == dch - 1))
                ks = evict.tile([P, T], BF16, name="k_ev", tag="kv_ev")
                qkv_evict(ps, ks, dch + m)
                nc.sync.dma_start(
                    out=dram_view(cck_in, m * P * T, [(T, P), (1, T)]),
                    in_=ks)
            for tm in range(cfg.tpr):  # v, token-major
                for f in range(cfg.dim // cfg.vf):
                    ps = psq.tile([P, cfg.vf], F32, name="ps_v", tag="ps_qkv")
                    for c in range(dch):
                        nc.tensor.matmul(
                            ps, lhsT=h1_sb[:, c, tm * P : (tm + 1) * P],
                            rhs=wq[c][:, 2 * cfg.dim + f * cfg.vf
                                      : 2 * cfg.dim + (f + 1) * cfg.vf],
                            start=(c == 0), stop=(c == dch - 1))
                    vs = evict.tile([P, cfg.vf], BF16, name="v_ev", tag="kv_ev")
                    nc.vector.tensor_copy(out=vs, in_=ps)  # v bias folded in later
                    nc.sync.dma_start(
                        out=dram_view(ccv_in, tm * P * cfg.dim + f * cfg.vf,
                                      [(cfg.dim, P), (1, cfg.vf)]),
                        in_=vs)
            for m in range(dch):  # q, feature-major (stays local)
                ps = psq.tile([P, T], F32, name="ps_q", tag="ps_qkv")
                for c in range(dch):
                    nc.tensor.matmul(ps, lhsT=wq[c][:, m * P : (m + 1) * P],
                                     rhs=h1_sb[:, c, :],
                                     start=(c == 0), stop=(c == dch - 1))
                qkv_evict(ps, qT_sb[:, m, :], m)

        LEVELS = ["qkv", "ag", "kvload", "attn", "proj", "fc1", "full"]
        _stop = LEVELS.index(cfg.stop_after)
        go = lambda ph: _stop >= LEVELS.index(ph)  # noqa: E731
        if go("ag") and not cfg.fake_ag:
            groups = [list(range(g * cfg.group, (g + 1) * cfg.group))
                      for g in range(cfg.n_cores // cfg.group)]
            nc.gpsimd.collective_compute(
                "AllGather", ALU.bypass,
                ins=[cck_in], outs=[cck_out], replica_groups=groups)
            nc.gpsimd.collective_compute(
                "AllGather", ALU.bypass,
                ins=[ccv_in], outs=[ccv_out], replica_groups=groups)

        # proj weights prefetch
        wp = []
        for c in range(dch):
            w = wprojp.tile([P, cfg.dim], BF16, name="wproj_t")
            nc.sync.dma_start(out=w, in_=wproj[c * P : (c + 1) * P, :])
            wp.append(w)

        # ---------------- Phase 2: attention ----------------
        oT_sb = opool.tile([P, dch, T], BF16, name="oT_sb")
        with tc.tile_pool(name="kTp", bufs=1) as kTp, \
             tc.tile_pool(name="vaugp", bufs=1) as vaugp, \
             tc.tile_pool(name="expp", bufs=3) as expp, \
             tc.tile_pool(name="psS", bufs=(2 if cfg.exp_batch == 1 else 1), space="PSUM") as psS, \
             tc.tile_pool(name="psO", bufs=1, space="PSUM") as psO, \
             tc.tile_pool(name="pspp", bufs=2, space="PSUM") as pspp:
            kT_sb = kTp.tile([P, cfg.group * dch, T], BF16, name="kT_sb")
            # c-outer: attention pair p consumes feature-chunk c==p of every
            # rank, so loading c-major lets pair 0 start after ~4 DMAs
            for c in range(dch if go("kvload") else 0):
                for r in range(cfg.group):
                    nc.sync.dma_start(
                        out=kT_sb[:, r * dch + c, :],
                        in_=dram_view(cck_in, c * P * T, [(T, P), (1, T)])
                        if cfg.fake_ag else
                        dram_view(cck_out, r * half + c * P * T,
                                  [(T, P), (1, T)]))
            v_aug = vaugp.tile([P, kt, cfg.heads, hd + 1], BF16, name="v_aug")
            nc.vector.memset(v_aug[:, :, :, hd : hd + 1], 1.0)
            for r in range(cfg.group if go("kvload") else 0):
                for jt in range(cfg.tpr):
                    vsrc = [(cfg.dim, P), (hd, cfg.heads), (1, hd)]
                    voff = jt * P * cfg.dim
                    nc.sync.dma_start(
                        out=v_aug[:, r * cfg.tpr + jt, :, :hd],
                        in_=dram_view(ccv_in, voff, vsrc) if cfg.fake_ag
                        else dram_view(ccv_out, r * half + voff, vsrc))

            x2_sb = x2p.tile([P, dch, T], F32, name="x2_sb", tag="x2", bufs=1)
            EB = cfg.exp_batch
            assert kt % EB == 0
            BK = max(T, 512)
            for p in range(cfg.pairs if go("attn") else 0):
                psO_A = psO.tile([P, T], F32, name="psO_A", tag="oA")
                psO_B = psO.tile([P, T], F32, name="psO_B", tag="oB")
                for J in range(kt // EB):
                    # the row-group-packed matmul pairs run concurrently on the
                    # PE: their outputs MUST land in different PSUM banks
                    # (same-bank concurrent writes are a hardware fault)
                    s2 = psS.tile([P, 2 * EB * BK], F32, name="s2")
                    s3 = s2.rearrange("p (b f) -> p b f", b=2 * EB)
                    for e in range(EB):
                        j = J * EB + e
                        r, jt = divmod(j, cfg.tpr)
                        kc = r * dch + p
                        kcol = slice(jt * P, (jt + 1) * P)
                        nc.tensor.matmul(s3[:, 2 * e, 0:T],
                                         lhsT=kT_sb[0:64, kc, kcol],
                                         rhs=qT_sb[0:64, p, :],
                                         start=True, stop=True)
                        nc.tensor.matmul(s3[:, 2 * e + 1, 0:T],
                                         lhsT=kT_sb[64:128, kc, kcol],
                                         rhs=qT_sb[64:128, p, :],
                                         start=True, stop=True)
                    if cfg.attn_parts == "qk":
                        continue
                    es = expp.tile([P, 2 * EB, T], BF16, name="es")
                    nc.scalar.activation(es, s3[:, :, 0:T], ACTF.Exp,
                                         scale=cfg.scale)
                    if cfg.attn_parts == "qkexp":
                        continue
                    for e in range(EB):
                        j = J * EB + e
                        nc.tensor.matmul(psO_A[0 : hd + 1, :],
                                         lhsT=v_aug[:, j, 2 * p, :],
                                         rhs=es[:, 2 * e, :],
                                         start=(j == 0), stop=(j == kt - 1))
                        nc.tensor.matmul(psO_B[0 : hd + 1, :],
                                         lhsT=v_aug[:, j, 2 * p + 1, :],
                                         rhs=es[:, 2 * e + 1, :],
                                         start=(j == 0), stop=(j == kt - 1))
                for h, pso, poff in (() if cfg.attn_parts != "full" else
                                     ((2 * p, psO_A, 0), (2 * p + 1, psO_B, 64))):
                    rrow = rows.tile([1, T], F32, name="rrow")
                    nc.vector.reciprocal(rrow, pso[hd : hd + 1, :])
                    rb = bcast.tile([P, T], F32, name="rb")
                    nc.gpsimd.partition_broadcast(rb[0:hd, :], rrow)
                    dst = oT_sb[poff : poff + hd, p, :]
                    nc.vector.tensor_mul(dst, pso[0:hd, :], rb[0:hd, :])
                    if qkvb_sb is not None:  # v bias: softmax rows sum to 1
                        nc.vector.tensor_scalar_add(
                            out=dst, in0=dst,
                            scalar1=qkvb_sb[poff : poff + hd,
                                            2 * dch + p : 2 * dch + p + 1])
                if not go("proj"):
                    continue
                # proj partial products for this pair's oT chunk, accumulated
                # into x2 in SBUF — fills PE idle under the ACT-bound window
                for m in range(dch):
                    pp = pspp.tile([P, T], F32, name="ps_pp")
                    nc.tensor.matmul(pp, lhsT=wp[p][:, m * P : (m + 1) * P],
                                     rhs=oT_sb[:, p, :], start=True, stop=True)
                    if p == 0:
                        nc.vector.tensor_copy(out=x2_sb[:, m, :], in_=pp)
                    else:
                        nc.vector.tensor_add(x2_sb[:, m, :], x2_sb[:, m, :], pp)

        # ---------------- proj residual finalize ----------------
        es_attn.close()  # qT no longer needed
        if go("proj"):
            for m in range(dch):
                if projb_sb is not None:
                    nc.vector.tensor_scalar_add(
                        out=x2_sb[:, m, :], in0=x2_sb[:, m, :],
                        scalar1=projb_sb[:, m : m + 1])
                nc.vector.tensor_add(x2_sb[:, m, :], x2_sb[:, m, :],
                                     xT_sb[:, m, :])

        es_proj.close()  # xT, oT, wproj no longer needed

        # ---------------- Phase 3: LN2 + fc1 + gelu + fc2 + residual ----------
        with tc.tile_pool(name="h2", bufs=1) as h2p, \
             tc.tile_pool(name="gpool", bufs=1) as gp, \
             tc.tile_pool(name="wfc1", bufs=dch) as wfc1p:
            h2_sb = h2p.tile([P, dch, T], BF16, name="h2_sb")
            g_sb = gp.tile([P, hch, T], BF16, name="g_sb")
            with tc.tile_pool(name="psln2", bufs=1, space="PSUM") as psln2, \
                 tc.tile_pool(name="psm", bufs=4, space="PSUM") as psm:
                if go("fc1"):
                    layer_norm(psln2, x2_sb, h2_sb, ln2g_sb, ln2b_sb)
                w1 = []
                for c in range(dch):
                    w = wfc1p.tile([P, cfg.hidden], BF16, name="wfc1_t")
                    nc.sync.dma_start(out=w, in_=wfc1[c * P : (c + 1) * P, :])
                    w1.append(w)
                for m in range(hch if go("fc1") else 0):
                    ps = psm.tile([P, T], F32, name="ps_fc1")
                    for c in range(dch):
                        nc.tensor.matmul(ps, lhsT=w1[c][:, m * P : (m + 1) * P],
                                         rhs=h2_sb[:, c, :],
                                         start=(c == 0), stop=(c == dch - 1))
                    nc.scalar.activation(
                        g_sb[:, m, :], ps, ACTF.Gelu,
                        bias=fc1b_sb[:, m : m + 1] if fc1b_sb is not None else 0.0)

            with tc.tile_pool(name="wfc2", bufs=3) as wfc2p, \
                 tc.tile_pool(name="psf2", bufs=1, space="PSUM") as psf2:
                acc = [psf2.tile([P, T], F32, name=f"ps_fc2_{m}", tag=f"acc{m}",
                                 bufs=1) for m in range(dch)]
                for c in range(hch if go("full") else 0):
                    w = wfc2p.tile([P, cfg.dim], BF16, name="wfc2_t")
                    nc.sync.dma_start(out=w, in_=wfc2[c * P : (c + 1) * P, :])
                    for m in range(dch):
                        nc.tensor.matmul(acc[m], lhsT=w[:, m * P : (m + 1) * P],
                                         rhs=g_sb[:, c, :],
                                         start=(c == 0), stop=(c == hch - 1))
                for m in range(dch if go("full") else 0):
                    ot = evict.tile([P, T], F32, name="out_ev", tag="out_ev")
                    if fc2b_sb is not None:
                        nc.vector.tensor_scalar_add(
                            out=ot, in0=acc[m], scalar1=fc2b_sb[:, m : m + 1])
                        nc.vector.tensor_add(ot, ot, x2_sb[:, m, :])
                    else:
                        nc.vector.tensor_add(ot, acc[m], x2_sb[:, m, :])
                    nc.sync.dma_start(out=outT[m * P : (m + 1) * P, :], in_=ot)

    return nc


# ----------------------------------------------------------------------------
# host wrapper
# ----------------------------------------------------------------------------
import time as _time

import jax
from jax.sharding import Mesh, PartitionSpec
from jax.experimental.shard_map import shard_map

from concourse import bacc
from concourse.bass2jax import (_bass_exec_p, install_neuronx_cc_hook,
                                partition_id_tensor)

_BF = ml_dtypes.bfloat16
_DIM, _HEADS, _HIDDEN = 1024, 16, 4096
_B, _N = 2, 2048
_GROUP, _NCORES = 4, 8
_T = _B * _N // _NCORES

_CACHE = {}


def _build_cfg(inputs):
    def nz(a):
        return bool(np.any(np.asarray(a)))

    return Cfg(
        dim=_DIM, heads=_HEADS, hidden=_HIDDEN, T=_T, group=_GROUP,
        n_cores=_NCORES,
        apply_ln1_gb=not (np.allclose(inputs["ln1_g"], 1.0)
                          and not nz(inputs["ln1_b"])),
        apply_ln2_gb=not (np.allclose(inputs["ln2_g"], 1.0)
                          and not nz(inputs["ln2_b"])),
        apply_qkv_bias=nz(inputs["qkv_b"]),
        apply_proj_bias=nz(inputs["proj_b"]),
        apply_fc1_bias=nz(inputs["fc1_b"]),
        apply_fc2_bias=nz(inputs["fc2_b"]),
    )


class _Runner:
    def __init__(self, cfg):
        import concourse.mybir as mybir

        self.cfg = cfg
        nc = bacc.Bacc("TRN2", target_bir_lowering=False, debug=False,
                       num_devices=_NCORES)
        build_block(nc, cfg)
        nc.compile()
        self.nc = nc
        install_neuronx_cc_hook()

        in_names, out_names, out_avals, zero_outs = [], [], [], []
        pid = nc.partition_id_tensor.name if nc.partition_id_tensor else None
        self.pid_name = pid
        for alloc in nc.m.functions[0].allocations:
            if not isinstance(alloc, mybir.MemoryLocationSet):
                continue
            name = alloc.memorylocations[0].name
            if alloc.kind == "ExternalInput":
                if name != pid:
                    in_names.append(name)
            elif alloc.kind == "ExternalOutput":
                out_names.append(name)
                shape = tuple(alloc.tensor_shape)
                dtype = mybir.dt.np(alloc.dtype)
                out_avals.append(jax.core.ShapedArray(shape, dtype))
                zero_outs.append(np.zeros(shape, dtype))
        assert out_names == ["outT"]
        self.in_names = in_names
        self.out_names = out_names
        self.out_avals = out_avals
        self.zero_outs = zero_outs
        self.ix_xT = in_names.index("xT")
        self.fns = {}

    def fn(self, n_iters):
        if n_iters in self.fns:
            return self.fns[n_iters]
        n_params = len(self.in_names)
        all_in = tuple(self.in_names + self.out_names
                       + ([self.pid_name] if self.pid_name else []))
        pid = self.pid_name
        out_avals = tuple(self.out_avals)
        out_names = tuple(self.out_names)
        nc = self.nc
        ix = self.ix_xT

        def _call(x, ins, zouts):
            operands = ins[:ix] + [x] + ins[ix + 1:] + zouts
            if pid:
                operands = operands + [partition_id_tensor()]
            outs = _bass_exec_p.bind(
                *operands, out_avals=out_avals, in_names=all_in,
                out_names=out_names, lowering_input_output_aliases=(),
                sim_require_finite=True, sim_require_nnan=True, nc=nc)
            return outs[0]

        def _body(*args):
            ins = list(args[:n_params])
            zouts = list(args[n_params:])
            x = ins[ix]
            if n_iters == 1:
                return (_call(x, ins, zouts),)
            # neuronx_cc_hook allows one bass_exec per XLA module: use scan
            import jax.lax as lax

            def step(carry, _):
                return _call(carry, ins, zouts), None

            x, _ = lax.scan(step, x, None, length=n_iters)
            return (x,)

        mesh = Mesh(np.asarray(jax.devices()[:_NCORES]), ("core",))
        specs = (PartitionSpec("core"),) * (n_params + 1)
        f = jax.jit(shard_map(_body, mesh=mesh, in_specs=specs,
                              out_specs=(PartitionSpec("core"),),
                              check_rep=False))
        self.fns[n_iters] = f
        return f

    def concat_inputs(self, inputs):
        x = np.asarray(inputs["x"], np.float32)
        shared = {
            "wqkv": np.ascontiguousarray(
                np.asarray(inputs["qkv_w"], np.float32)).astype(_BF),
            "wproj": np.asarray(inputs["proj_w"], np.float32).astype(_BF),
            "wfc1": np.asarray(inputs["fc1_w"], np.float32).astype(_BF),
            "wfc2": np.asarray(inputs["fc2_w"], np.float32).astype(_BF),
        }
        cfg = self.cfg
        for flag, names in (
            (cfg.apply_ln1_gb, ("ln1_g", "ln1_b")),
            (cfg.apply_ln2_gb, ("ln2_g", "ln2_b")),
            (cfg.apply_qkv_bias, ("qkv_b",)),
            (cfg.apply_proj_bias, ("proj_b",)),
            (cfg.apply_fc1_bias, ("fc1_b",)),
            (cfg.apply_fc2_bias, ("fc2_b",)),
        ):
            if flag:
                for n in names:
                    shared[n] = np.asarray(inputs[n], np.float32)
        per_core = []
        for c in range(_NCORES):
            b, q = divmod(c, _GROUP)
            xc = x[b, q * _T : (q + 1) * _T, :]
            m = {"xT": np.ascontiguousarray(xc.T), **shared}
            per_core.append([m[nm] for nm in self.in_names])
        concat = [np.concatenate([per_core[c][i] for c in range(_NCORES)], axis=0)
                  for i in range(len(self.in_names))]
        concat += [np.zeros((_NCORES * z.shape[0], *z.shape[1:]), z.dtype)
                   for z in self.zero_outs]
        return concat

    def run(self, inputs, n_iters=1):
        args = self.concat_inputs(inputs)
        out = self.fn(n_iters)(*args)
        jax.block_until_ready(out)
        return np.asarray(out[0])

    def measure_ns(self, inputs, n=50, reps=3):
        """Queued-chain wall estimate: upper bound incl. per-dispatch RPC."""
        args = self.concat_inputs(inputs)
        f = self.fn(1)
        lowered = f.lower(*args)
        compiled = lowered.compile()
        shardings = compiled.input_shardings[0]
        dev = [jax.device_put(a, s) for a, s in zip(args, shardings)]
        jax.block_until_ready(dev)
        ix = self.ix_xT
        out = f(*dev)
        jax.block_until_ready(out)
        best = None
        for _ in range(reps):
            x = dev[ix]
            t0 = _time.perf_counter()
            for _i in range(n):
                out = f(*(dev[:ix] + [x] + dev[ix + 1:]))
                x = out[0]
            jax.block_until_ready(out)
            est = (_time.perf_counter() - t0) / n
            best = est if best is None else min(best, est)
        return best * 1e9


def _get_runner(inputs):
    cfg = _build_cfg(inputs)
    key = (cfg.apply_ln1_gb, cfg.apply_ln2_gb, cfg.apply_qkv_bias,
           cfg.apply_proj_bias, cfg.apply_fc1_bias, cfg.apply_fc2_bias)
    if key not in _CACHE:
        _CACHE[key] = _Runner(cfg)
    return _CACHE[key]


def kernel(**inputs) -> np.ndarray:
    r = _get_runner(inputs)
    flat = r.run(inputs)  # [8*DIM, T] stacked per-core outT
    out = np.empty((_B, _N, _DIM), np.float32)
    for c in range(_NCORES):
        b, q = divmod(c, _GROUP)
        out[b, q * _T : (q + 1) * _T, :] = flat[c * _DIM : (c + 1) * _DIM, :].T
    return out


def measure_hw_time_ns(**inputs) -> float:
    """Estimate per-execution device time by differencing chained runs."""
    return _get_runner(inputs).measure_ns(inputs)



# revision 44
# speedup vs baseline: 1.2660x; 1.2660x over previous
"""nn_Block dense_transformer kernel for 8 TRN2 NeuronCores.

Self-contained: builds the Bass/Tile program, shards inputs across 8 cores
(sequence-parallel, 4 cores per batch element), runs via a cached jitted
shard_map over jax's neuron devices, gathers the full output.

Numerics: fp8-e4m3 DoubleRow matmuls for qkv / QK / AV / proj / fc1 (fc1 with
scaled-residual error correction), bf16 fc2, exp->fp8 softmax with a
softmax-invariant score shift; part of the exp work runs on DVE via a
Schraudolph-style uint8 bit trick. Validated ~5.7e-3 rel err vs f32 ref.
"""
import os
import numpy as np
import ml_dtypes


from contextlib import ExitStack
from dataclasses import dataclass

import concourse.bass as bass
import concourse.mybir as mybir
import concourse.tile as tile

F32 = mybir.dt.float32
F32R = mybir.dt.float32r
BF16 = mybir.dt.bfloat16
F8 = mybir.dt.float8e4
U8 = mybir.dt.uint8
P = 128
ALU = mybir.AluOpType
ACTF = mybir.ActivationFunctionType
DR = mybir.MatmulPerfMode.DoubleRow

LOG2E = 1.4426950408889634
RES_K = 32.0   # fc1 residual pre-scale


@dataclass
class Cfg:
    dim: int = 1024
    heads: int = 16
    hd: int = 64
    hidden: int = 4096
    T: int = 512          # tokens per core
    group: int = 4        # cores per batch group
    n_cores: int = 8
    eps: float = 1e-5
    shift: float = 2.6    # softmax-invariant score shift (fit e^s into e4m3)
    # build-time specialization flags (host inspects actual input values)
    apply_ln1_gb: bool = False
    apply_ln2_gb: bool = False
    apply_qkv_bias: bool = False
    apply_proj_bias: bool = False
    apply_fc1_bias: bool = False
    apply_fc2_bias: bool = False
    fake_ag: bool = False  # timing-only: skip collective, read own kv as all ranks
    # j-indices (within each pair's 16 j-steps) whose exp runs on DVE via the
    # Schraudolph uint8 trick; the rest run on ACT (native exp)
    dve_js: tuple = (1, 3, 6, 9, 12, 14)
    schr_round: float = 0.0  # probe-verified: DVE f32->u8 rounds-to-nearest
    stop_after: str = "full"  # bisect: qkv|ag|kvload|attn|proj|fc1|full

    @property
    def dch(self):
        return self.dim // P

    @property
    def hch(self):
        return self.hidden // P

    @property
    def kt(self):
        return (self.group * self.T) // P

    @property
    def tpr(self):
        return self.T // P

    @property
    def pairs(self):
        return self.heads // 2

    @property
    def vf(self):  # free-dim chunk for the v matmul
        return min(512, self.dim)

    @property
    def schr_b(self):
        # uint8(LOG2E * raw_score + schr_b) bitcast e4m3 ~= exp(score/8 - shift)
        return 56.0 - 8.0 * LOG2E * self.shift - 8.0 * 0.043677 + self.schr_round


def build_block(nc: bass.Bass, cfg: Cfg):
    dch, hch, kt, T, hd = cfg.dch, cfg.hch, cfg.kt, cfg.T, cfg.hd

    xT = nc.dram_tensor("xT", [cfg.dim, T], F32, kind="ExternalInput").ap()
    wqkv = nc.dram_tensor("wqkv", [cfg.dim, 3 * cfg.dim], F8, kind="ExternalInput").ap()
    wproj = nc.dram_tensor("wproj", [cfg.dim, cfg.dim], F8, kind="ExternalInput").ap()
    wfc1 = nc.dram_tensor("wfc1", [cfg.dim, cfg.hidden], F8, kind="ExternalInput").ap()
    wfc1r = nc.dram_tensor("wfc1r", [cfg.dim, cfg.hidden], F8, kind="ExternalInput").ap()
    wfc2 = nc.dram_tensor("wfc2", [cfg.hidden, cfg.dim], BF16, kind="ExternalInput").ap()
    outT = nc.dram_tensor("outT", [cfg.dim, T], F32, kind="ExternalOutput").ap()

    ln1_g = ln1_b = ln2_g = ln2_b = None
    if cfg.apply_ln1_gb:
        ln1_g = nc.dram_tensor("ln1_g", [cfg.dim], F32, kind="ExternalInput").ap()
        ln1_b = nc.dram_tensor("ln1_b", [cfg.dim], F32, kind="ExternalInput").ap()
    if cfg.apply_ln2_gb:
        ln2_g = nc.dram_tensor("ln2_g", [cfg.dim], F32, kind="ExternalInput").ap()
        ln2_b = nc.dram_tensor("ln2_b", [cfg.dim], F32, kind="ExternalInput").ap()
    qkv_b = proj_b = fc1_b = fc2_b = None
    if cfg.apply_qkv_bias:
        qkv_b = nc.dram_tensor("qkv_b", [3 * cfg.dim], F32, kind="ExternalInput").ap()
    if cfg.apply_proj_bias:
        proj_b = nc.dram_tensor("proj_b", [cfg.dim], F32, kind="ExternalInput").ap()
    if cfg.apply_fc1_bias:
        fc1_b = nc.dram_tensor("fc1_b", [cfg.hidden], F32, kind="ExternalInput").ap()
    if cfg.apply_fc2_bias:
        fc2_b = nc.dram_tensor("fc2_b", [cfg.dim], F32, kind="ExternalInput").ap()

    # collective bounce buffers, fp8 (k first so its AllGather can complete and
    # feed QK while v's AllGather still runs)
    half = cfg.dim * T
    cck_in = nc.dram_tensor("cck_in", [half], F8, kind="Internal").ap()
    cck_out = nc.dram_tensor("cck_out", [cfg.group * half], F8, kind="Internal").ap()
    ccv_in = nc.dram_tensor("ccv_in", [half], F8, kind="Internal").ap()
    ccv_out = nc.dram_tensor("ccv_out", [cfg.group * half], F8, kind="Internal").ap()

    def dram_view(ap, off, shape_strides):
        return bass.AP(tensor=ap.tensor, offset=ap.offset + off,
                       ap=[[s, n] for s, n in shape_strides])

    LEVELS = ["qkv", "ag", "kvload", "attn", "proj", "ln2", "fc1", "full"]
    _stop = LEVELS.index(cfg.stop_after)
    go = lambda ph: _stop >= LEVELS.index(ph)  # noqa: E731

    with tile.TileContext(nc) as tc, ExitStack() as ctx:
        const = ctx.enter_context(tc.tile_pool(name="const", bufs=1))
        rows = ctx.enter_context(tc.tile_pool(name="rows", bufs=2))
        bcast = ctx.enter_context(tc.tile_pool(name="bcast", bufs=2))
        evict = ctx.enter_context(tc.tile_pool(name="evict", bufs=3))
        tmp = ctx.enter_context(tc.tile_pool(name="tmp", bufs=2))
        # pool release is strict LIFO: open long-lived pools first
        x2p = ctx.enter_context(tc.tile_pool(name="x2p", bufs=1))
        es_qr = ExitStack()     # qr: lives until end of attention
        es_proj = ExitStack()   # freed after proj: xT, oT, wproj
        es_q = ExitStack()      # qT_8: freed after the qr remap
        wfc1p = ctx.enter_context(tc.tile_pool(name="wfc1", bufs=1))
        qrp = es_qr.enter_context(tc.tile_pool(name="qrp", bufs=1))
        xpool = es_proj.enter_context(tc.tile_pool(name="xpool", bufs=1))
        opool = es_proj.enter_context(tc.tile_pool(name="opool", bufs=1))
        wprojp = es_proj.enter_context(tc.tile_pool(name="wproj", bufs=4))
        qpool = es_q.enter_context(tc.tile_pool(name="qpool", bufs=1))

        ones_bf = const.tile([P, 1], BF16, tag="onesbf", bufs=1)
        nc.vector.memset(ones_bf, 1.0)
        eps_t = const.tile([1, 1], F32, tag="eps", bufs=1)
        nc.vector.memset(eps_t, cfg.eps)
        nshift_t = const.tile([P, 1], F32, tag="nshift", bufs=1)
        nc.vector.memset(nshift_t, -cfg.shift)

        xT_sb = xpool.tile([P, dch, T], F32, name="xT_sb", tag="xT", bufs=1)
        for c in range(0, dch, 2):
            nc.sync.dma_start(
                out=xT_sb[:, c : c + 2, :],
                in_=dram_view(xT, c * P * T, [(T, P), (P * T, 2), (1, T)]))

        def load_cols(vec, nch, nm):
            t = const.tile([P, nch], F32, name=nm, tag=nm, bufs=1)
            nc.sync.dma_start(out=t, in_=vec.rearrange("(c p) -> p c", p=P))
            return t

        ln1g_sb = load_cols(ln1_g, dch, "ln1g") if ln1_g is not None else None
        ln1b_sb = load_cols(ln1_b, dch, "ln1b") if ln1_b is not None else None
        ln2g_sb = load_cols(ln2_g, dch, "ln2g") if ln2_g is not None else None
        ln2b_sb = load_cols(ln2_b, dch, "ln2b") if ln2_b is not None else None
        qkvb_sb = load_cols(qkv_b, 3 * dch, "qkvb") if qkv_b is not None else None
        projb_sb = load_cols(proj_b, dch, "projb") if proj_b is not None else None
        fc1b_sb = load_cols(fc1_b, hch, "fc1b") if fc1_b is not None else None
        fc2b_sb = load_cols(fc2_b, dch, "fc2b") if fc2_b is not None else None

        # ---- LayerNorm stats over features (feature-major data) ----
        # returns (r1b, r2b) broadcast rows: normalized = x*r1b + r2b
        def ln_stats(psln, x_sb):
            sum_x = psln.tile([1, T], F32, name="sum_x", tag="sumx", bufs=1)
            sum_sq = psln.tile([1, T], F32, name="sum_sq", tag="sumsq", bufs=1)
            for c in range(dch):
                eng = nc.gpsimd if c % 2 else nc.vector
                eng2 = nc.vector if c % 2 else nc.gpsimd
                xb = tmp.tile([P, T], BF16, name="xb", tag="xb")
                eng.tensor_copy(out=xb, in_=x_sb[:, c, :])
                xsq = tmp.tile([P, T], BF16, name="xsq", tag="xsq")
                eng2.tensor_mul(xsq, x_sb[:, c, :], x_sb[:, c, :])
                nc.tensor.matmul(sum_x, lhsT=ones_bf, rhs=xb,
                                 start=(c == 0), stop=(c == dch - 1))
                nc.tensor.matmul(sum_sq, lhsT=ones_bf, rhs=xsq,
                                 start=(c == 0), stop=(c == dch - 1))
            inv_d = 1.0 / cfg.dim
            m_row = rows.tile([1, T], F32, name="m_row", bufs=1)
            var_row = rows.tile([1, T], F32, name="var_row", bufs=1)
            nc.vector.tensor_scalar(out=m_row, in0=sum_x, scalar1=inv_d,
                                    scalar2=None, op0=ALU.mult)
            nc.vector.tensor_mul(var_row, m_row, m_row)           # m^2
            nc.vector.scalar_tensor_tensor(                        # sum_sq/D - m^2
                out=var_row, in0=sum_sq, scalar=inv_d, in1=var_row,
                op0=ALU.mult, op1=ALU.subtract)
            lnv = rows.tile([1, T], F32, name="lnv", bufs=1)
            rstd = rows.tile([1, T], F32, name="rstd", bufs=1)
            nc.scalar.activation(lnv, var_row, ACTF.Ln, bias=eps_t)
            nc.scalar.activation(rstd, lnv, ACTF.Exp, scale=-0.5)
            r2 = rows.tile([1, T], F32, name="r2", bufs=1)         # -m * rstd
            nc.vector.scalar_tensor_tensor(
                out=r2, in0=m_row, scalar=-1.0, in1=rstd,
                op0=ALU.mult, op1=ALU.mult)
            r1b = bcast.tile([P, T], F32, name="r1b", bufs=1)
            r2b = bcast.tile([P, T], F32, name="r2b", bufs=1)
            nc.gpsimd.partition_broadcast(r1b, rstd)
            nc.gpsimd.partition_broadcast(r2b, r2)
            return r1b, r2b

        # ---------------- Phase 1: LN1 + QKV(fp8 DR) + AllGather ----------------
        qT_8 = qpool.tile([P, dch, T], F8, name="qT_8")
        with tc.tile_pool(name="h1", bufs=1) as h1p, \
             tc.tile_pool(name="wqkv", bufs=4) as wqkvp, \
             tc.tile_pool(name="psln1", bufs=1, space="PSUM") as psln1, \
             tc.tile_pool(name="psqkv", bufs=4, space="PSUM") as psq:
            h1_8 = h1p.tile([P, dch, T], F8, name="h1_8")
            r1b, r2b = ln_stats(psln1, xT_sb)
            for c in range(dch):
                eng = nc.gpsimd if c % 2 else nc.vector
                t0 = tmp.tile([P, T], F32, name="ln_t0", tag="t0")
                eng.tensor_mul(t0, xT_sb[:, c, :], r1b)
                if ln1g_sb is not None:
                    t1 = tmp.tile([P, T], F32, name="ln_t1", tag="t1")
                    eng.tensor_add(t1, t0, r2b)
                    eng.tensor_scalar(
                        out=h1_8[:, c, :], in0=t1,
                        scalar1=ln1g_sb[:, c : c + 1], scalar2=ln1b_sb[:, c : c + 1],
                        op0=ALU.mult, op1=ALU.add)
                else:
                    eng.tensor_add(h1_8[:, c, :], t0, r2b)

            wq = []
            for u in range(dch // 2):
                w = wqkvp.tile([P, 2, 3 * cfg.dim], F8, name="wqkv_t")
                nc.sync.dma_start(
                    out=w, in_=dram_view(
                        wqkv, 2 * u * P * 3 * cfg.dim,
                        [(3 * cfg.dim, P), (P * 3 * cfg.dim, 2), (1, 3 * cfg.dim)]))
                wq.append(w)

            def qkv_evict(ps, dst, col_idx):
                # on ACT: it is idle during this phase while DVE is loaded
                if qkvb_sb is not None:
                    nc.scalar.activation(dst, ps, ACTF.Identity,
                                         bias=qkvb_sb[:, col_idx : col_idx + 1])
                else:
                    nc.scalar.copy(dst, ps)

            # k first (feeds the AllGather + kr loads), then q (gates the qr
            # remap and so the attention start), then v (only needed at the
            # first AV, a few us into attention). k/q evict on ACT, v on DVE.
            kfull = evict.tile([P, dch, T], F8, name="kfull", tag="kfull", bufs=1)
            for m in range(dch):  # k, feature-major, fp8
                ps = psq.tile([P, T], F32, name="ps_qk", tag="ps_qkv")
                for u in range(dch // 2):
                    nc.tensor.matmul(
                        ps, lhsT=wq[u][:, :, (dch + m) * P : (dch + m + 1) * P],
                        rhs=h1_8[:, 2 * u : 2 * u + 2, :],
                        start=(u == 0), stop=(u == dch // 2 - 1), perf_mode=DR)
                qkv_evict(ps, kfull[:, m, :], dch + m)
            nc.sync.dma_start(
                out=dram_view(cck_in, 0, [(T, P), (P * T, dch), (1, T)]),
                in_=kfull)
            for m in range(dch):  # q, feature-major (stays local), fp8
                ps = psq.tile([P, T], F32, name="ps_q", tag="ps_qkv")
                for u in range(dch // 2):
                    nc.tensor.matmul(
                        ps, lhsT=wq[u][:, :, m * P : (m + 1) * P],
                        rhs=h1_8[:, 2 * u : 2 * u + 2, :],
                        start=(u == 0), stop=(u == dch // 2 - 1), perf_mode=DR)
                qkv_evict(ps, qT_8[:, m, :], m)
            vfull = evict.tile([P, cfg.tpr, 2, cfg.vf], F8, name="vfull",
                               tag="vfull", bufs=1)
            for tm in range(cfg.tpr):  # v, token-major, fp8
                for f in range(cfg.dim // cfg.vf):
                    ps = psq.tile([P, cfg.vf], F32, name="ps_v", tag="ps_qkv")
                    for u in range(dch // 2):
                        nc.tensor.matmul(
                            ps, lhsT=h1_8[:, 2 * u : 2 * u + 2, tm * P : (tm + 1) * P],
                            rhs=wq[u][:, :, 2 * cfg.dim + f * cfg.vf
                                      : 2 * cfg.dim + (f + 1) * cfg.vf],
                            start=(u == 0), stop=(u == dch // 2 - 1), perf_mode=DR)
                    # v bias folded in after the softmax (rows sum to 1)
                    nc.vector.tensor_copy(out=vfull[:, tm, f, :], in_=ps)
            nc.sync.dma_start(
                out=dram_view(ccv_in, 0,
                              [(cfg.dim, P), (P * cfg.dim, cfg.tpr), (1, cfg.dim)]),
                in_=vfull)

        if go("ag") and not cfg.fake_ag:
            groups = [list(range(g * cfg.group, (g + 1) * cfg.group))
                      for g in range(cfg.n_cores // cfg.group)]
            nc.gpsimd.collective_compute(
                "AllGather", ALU.bypass,
                ins=[cck_in], outs=[cck_out], replica_groups=groups)
            nc.gpsimd.collective_compute(
                "AllGather", ALU.bypass,
                ins=[ccv_in], outs=[ccv_out], replica_groups=groups)

        # proj + fc1 weight tiles (fp8, c-pair layout for DoubleRow); allocated
        # now, but DMA issue is deferred into the attention loop so the
        # attention-critical kr/v loads hit the DMA path first
        wp = [wprojp.tile([P, 2, cfg.dim], F8, name="wproj_t")
              for _ in range(dch // 2)]
        w1m = [wfc1p.tile([P, 2, cfg.hidden], F8, name="wfc1_m", tag=f"w1m{u}",
                          bufs=1) for u in range(dch // 2)]
        w1r = [wfc1p.tile([P, 2, cfg.hidden], F8, name="wfc1_r", tag=f"w1r{u}",
                          bufs=1) for u in range(dch // 2)]

        def pair_view(nrow):
            return [(nrow, P), (P * nrow, 2), (1, nrow)]

        _deferred = []
        for u in range(dch // 2):
            _deferred.append((wp[u], dram_view(
                wproj, 2 * u * P * cfg.dim, pair_view(cfg.dim))))
        for u in range(dch // 2):
            _deferred.append((w1m[u], dram_view(
                wfc1, 2 * u * P * cfg.hidden, pair_view(cfg.hidden))))
            _deferred.append((w1r[u], dram_view(
                wfc1r, 2 * u * P * cfg.hidden, pair_view(cfg.hidden))))

        # ---------------- Phase 2: attention ----------------
        # q remap to DoubleRow layout [32, half, head, T] via 4 SBUF DMAs
        qr = qrp.tile([32, 2, cfg.heads, T], F8, name="qr")
        for hp in range(2):
            for hf in range(2):
                p0 = hp * 64 + hf * 32
                nc.sync.dma_start(
                    out=qr[:, hf, hp : cfg.heads : 2, :],
                    in_=qT_8[p0 : p0 + 32, :, :])
        es_q.close()  # qT_8 freed

        oT_8 = opool.tile([P, dch, T], F8, name="oT_8")
        with tc.tile_pool(name="krp", bufs=2) as krp, \
             tc.tile_pool(name="vaugp", bufs=1) as vaugp, \
             tc.tile_pool(name="expp", bufs=3) as expp, \
             tc.tile_pool(name="psS", bufs=3, space="PSUM") as psS, \
             tc.tile_pool(name="psO", bufs=1, space="PSUM") as psO:
            def load_kr(p):
                # k for pair p, DoubleRow layout [32, head(2), half, keys];
                # (head, half) merge on both sides -> one DMA per rank
                kr = krp.tile([32, 2, 2, cfg.group * T], F8, name="kr", tag="kr")
                for r in range(cfg.group):
                    ksrc = [(T, 32), (32 * T, 4), (1, T)]
                    koff = 2 * p * hd * T
                    nc.sync.dma_start(
                        out=kr[:, :, :, r * T : (r + 1) * T],
                        in_=dram_view(cck_in, koff, ksrc) if cfg.fake_ag
                        else dram_view(cck_out, r * half + koff, ksrc))
                return kr

            v_aug = vaugp.tile([P, kt, cfg.heads, hd + 1], F8, name="v_aug")
            nc.vector.memset(v_aug[:, :, :, hd : hd + 1], 1.0)
            kr_next = load_kr(0) if go("attn") else None  # before the v stream
            for r in range(cfg.group if go("kvload") else 0):
                for jt in range(cfg.tpr):
                    vsrc = [(cfg.dim, P), (hd, cfg.heads), (1, hd)]
                    voff = jt * P * cfg.dim
                    nc.sync.dma_start(
                        out=v_aug[:, r * cfg.tpr + jt, :, :hd],
                        in_=dram_view(ccv_in, voff, vsrc) if cfg.fake_ag
                        else dram_view(ccv_out, r * half + voff, vsrc))

            x2_sb = x2p.tile([P, dch, T], F32, name="x2_sb", tag="x2", bufs=1)
            for p in range(cfg.pairs if go("attn") else 0):
                kr = kr_next
                if p + 1 < cfg.pairs:
                    kr_next = load_kr(p + 1)
                # drip the deferred proj/fc1 weight loads into the late
                # (DMA-idle) half of the attention window
                if p >= 4:
                    for w, src in _deferred[4 * (p - 4) : 4 * (p - 3)]:
                        nc.sync.dma_start(out=w, in_=src)

                psO_A = psO.tile([hd + 1, T], F32, name="psO_A", tag="oA")
                psO_B = psO.tile([hd + 1, T], F32, name="psO_B", tag="oB")
                for uj in range(kt // 2):
                    es = expp.tile([P, 2, 2, T], F8, name="es", tag="es")
                    es_u8 = es.bitcast(U8)
                    for i in range(2):
                        j = 2 * uj + i
                        s = psS.tile([P, 2, T], F32, name="s_qk", tag="s")
                        kcol = slice(j * P, (j + 1) * P)
                        for hh in range(2):
                            nc.tensor.matmul(
                                s[:, hh, :], lhsT=kr[:, hh, :, kcol],
                                rhs=qr[:, :, 2 * p + hh, :],
                                start=True, stop=True, perf_mode=DR)
                        if j in cfg.dve_js:
                            # Schraudolph exp on DVE: uint8(A*raw+B) bits = e4m3
                            nc.vector.tensor_scalar(
                                out=es_u8[:, i, :, :], in0=s,
                                scalar1=LOG2E, scalar2=cfg.schr_b,
                                op0=ALU.mult, op1=ALU.add)
                        else:
                            nc.scalar.activation(
                                es[:, i, :, :], s, ACTF.Exp,
                                scale=0.125, bias=nshift_t)
                    nc.tensor.matmul(
                        psO_A, lhsT=v_aug[:, 2 * uj : 2 * uj + 2, 2 * p, :],
                        rhs=es[:, :, 0, :],
                        start=(uj == 0), stop=(uj == kt // 2 - 1), perf_mode=DR)
                    nc.tensor.matmul(
                        psO_B, lhsT=v_aug[:, 2 * uj : 2 * uj + 2, 2 * p + 1, :],
                        rhs=es[:, :, 1, :],
                        start=(uj == 0), stop=(uj == kt // 2 - 1), perf_mode=DR)
                for h, pso, poff in ((2 * p, psO_A, 0), (2 * p + 1, psO_B, 64)):
                    rrow = rows.tile([1, T], F32, name="rrow")
                    nc.vector.reciprocal(rrow, pso[hd : hd + 1, :])
                    rb = bcast.tile([P, T], F32, name="rb")
                    nc.gpsimd.partition_broadcast(rb[0:hd, :], rrow)
                    dst = oT_8[poff : poff + hd, p, :]
                    nc.vector.tensor_mul(dst, pso[0:hd, :], rb[0:hd, :])
                    if qkvb_sb is not None:  # v bias: softmax rows sum to 1
                        nc.vector.tensor_scalar_add(
                            out=dst, in0=dst,
                            scalar1=qkvb_sb[poff : poff + hd,
                                            2 * dch + p : 2 * dch + p + 1])

        # ---------------- proj (fp8 DR, PSUM-accumulated) + residual ----------
        if go("proj"):
            with tc.tile_pool(name="pspp", bufs=2, space="PSUM") as pspp:
                for m in range(dch):
                    pp = pspp.tile([P, T], F32, name="ps_pp")
                    for u in range(dch // 2):
                        nc.tensor.matmul(
                            pp, lhsT=wp[u][:, :, m * P : (m + 1) * P],
                            rhs=oT_8[:, 2 * u : 2 * u + 2, :],
                            start=(u == 0), stop=(u == dch // 2 - 1), perf_mode=DR)
                    if projb_sb is not None:
                        nc.vector.tensor_scalar_add(
                            out=pp, in0=pp, scalar1=projb_sb[:, m : m + 1])
                    nc.vector.tensor_add(x2_sb[:, m, :], pp, xT_sb[:, m, :])

        es_proj.close()  # xT, oT, wproj no longer needed
        es_qr.close()    # qr freed

        # ------------- Phase 3: LN2 + fc1(fp8 DR, corrected) + gelu + fc2 -----
        gpool = ctx.enter_context(tc.tile_pool(name="gpool", bufs=1))
        g_sb = gpool.tile([P, hch, T], BF16, name="g_sb")
        with tc.tile_pool(name="h2", bufs=1) as h2p:
            h2_8 = h2p.tile([P, dch, T], F8, name="h2_8")
            h2d_8 = h2p.tile([P, dch, T], F8, name="h2d_8")
            l2_8 = h2p.tile([P, dch, T], F8, name="l2_8")
            with tc.tile_pool(name="psln2", bufs=1, space="PSUM") as psln2:
                if go("ln2"):
                    r1b, r2b = ln_stats(psln2, x2_sb)
                    for c in range(dch):
                        eng = nc.gpsimd if c % 2 else nc.vector
                        t0 = tmp.tile([P, T], F32, name="l2t0", tag="t0")
                        t1 = tmp.tile([P, T], F32, name="l2t1", tag="t1")
                        eng.tensor_mul(t0, x2_sb[:, c, :], r1b)
                        if ln2g_sb is not None:
                            eng.tensor_add(t1, t0, r2b)
                            eng.tensor_scalar(
                                out=t1, in0=t1,
                                scalar1=ln2g_sb[:, c : c + 1],
                                scalar2=ln2b_sb[:, c : c + 1],
                                op0=ALU.mult, op1=ALU.add)
                        else:
                            eng.tensor_add(t1, t0, r2b)
                        # fc1 correction operands, pre-scaled so all three
                        # matmul series accumulate into ONE psum bank:
                        #   h8*W8 + fp8(t1/K)*RW8 + fp8(t1-h8)*W8
                        nc.scalar.copy(h2_8[:, c, :], t1)
                        nc.gpsimd.tensor_scalar(
                            out=h2d_8[:, c, :], in0=t1, scalar1=1.0 / RES_K,
                            scalar2=None, op0=ALU.mult)
                        nc.vector.tensor_tensor(
                            out=l2_8[:, c, :], in0=t1, in1=h2_8[:, c, :],
                            op=ALU.subtract)

            with tc.tile_pool(name="psm", bufs=4, space="PSUM") as psm:
                for m in range(hch if go("fc1") else 0):
                    ps = psm.tile([P, T], F32, name="ps_fc1", tag="fm")
                    for u in range(dch // 2):
                        nc.tensor.matmul(
                            ps, lhsT=w1m[u][:, :, m * P : (m + 1) * P],
                            rhs=h2_8[:, 2 * u : 2 * u + 2, :],
                            start=(u == 0), stop=False, perf_mode=DR)
                    for u in range(dch // 2):
                        nc.tensor.matmul(
                            ps, lhsT=w1r[u][:, :, m * P : (m + 1) * P],
                            rhs=h2d_8[:, 2 * u : 2 * u + 2, :],
                            start=False, stop=False, perf_mode=DR)
                    for u in range(dch // 2):
                        nc.tensor.matmul(
                            ps, lhsT=w1m[u][:, :, m * P : (m + 1) * P],
                            rhs=l2_8[:, 2 * u : 2 * u + 2, :],
                            start=False, stop=(u == dch // 2 - 1), perf_mode=DR)
                    nc.scalar.activation(
                        g_sb[:, m, :], ps, ACTF.Gelu,
                        bias=fc1b_sb[:, m : m + 1] if fc1b_sb is not None else 0.0)

        with tc.tile_pool(name="wfc2", bufs=3) as wfc2p, \
             tc.tile_pool(name="psf2", bufs=1, space="PSUM") as psf2:
            acc = [psf2.tile([P, T], F32, name=f"ps_fc2_{m}", tag=f"acc{m}",
                             bufs=1) for m in range(dch)]
            for c4 in range(0, hch if go("full") else 0, 4):
                w = wfc2p.tile([P, 4, cfg.dim], BF16, name="wfc2_t")
                nc.sync.dma_start(
                    out=w, in_=dram_view(
                        wfc2, c4 * P * cfg.dim,
                        [(cfg.dim, P), (P * cfg.dim, 4), (1, cfg.dim)]))
                for ci in range(4):
                    c = c4 + ci
                    for m in range(dch):
                        nc.tensor.matmul(
                            acc[m], lhsT=w[:, ci, m * P : (m + 1) * P],
                            rhs=g_sb[:, c, :],
                            start=(c == 0), stop=(c == hch - 1))
            # final residual add in place into x2, then store in two DMAs
            for m in range(dch if go("full") else 0):
                if fc2b_sb is not None:
                    nc.vector.tensor_scalar_add(
                        out=acc[m], in0=acc[m], scalar1=fc2b_sb[:, m : m + 1])
                nc.vector.tensor_add(x2_sb[:, m, :], acc[m], x2_sb[:, m, :])
                if go("full") and m % (dch // 2) == dch // 2 - 1:
                    m0 = m - dch // 2 + 1
                    nc.sync.dma_start(
                        out=dram_view(outT, m0 * P * T,
                                      [(T, P), (P * T, dch // 2), (1, T)]),
                        in_=x2_sb[:, m0 : m + 1, :])

    return nc


# ----------------------------------------------------------------------------
# host wrapper
# ----------------------------------------------------------------------------
import time as _time

import jax
from jax.sharding import Mesh, PartitionSpec
from jax.experimental.shard_map import shard_map

from concourse import bacc
from concourse.bass2jax import (_bass_exec_p, install_neuronx_cc_hook,
                                partition_id_tensor)

_BF = ml_dtypes.bfloat16
_F8 = ml_dtypes.float8_e4m3
_DIM, _HEADS, _HIDDEN = 1024, 16, 4096
_B, _N = 2, 2048
_GROUP, _NCORES = 4, 8
_T = _B * _N // _NCORES

_CACHE = {}


def _build_cfg(inputs):
    def nz(a):
        return bool(np.any(np.asarray(a)))

    return Cfg(
        dim=_DIM, heads=_HEADS, hidden=_HIDDEN, T=_T, group=_GROUP,
        n_cores=_NCORES,
        apply_ln1_gb=not (np.allclose(inputs["ln1_g"], 1.0)
                          and not nz(inputs["ln1_b"])),
        apply_ln2_gb=not (np.allclose(inputs["ln2_g"], 1.0)
                          and not nz(inputs["ln2_b"])),
        apply_qkv_bias=nz(inputs["qkv_b"]),
        apply_proj_bias=nz(inputs["proj_b"]),
        apply_fc1_bias=nz(inputs["fc1_b"]),
        apply_fc2_bias=nz(inputs["fc2_b"]),
    )


def _prefer_act_tables(arch):
    """No-op: reordering the cached act-table dict desyncs act_func_set_id
    (a positional index into act_info.json) from the walrus-side mapping."""


class _Runner:
    def __init__(self, cfg):
        import concourse.mybir as mybir

        self.cfg = cfg
        nc = bacc.Bacc("TRN2", target_bir_lowering=False, debug=False,
                       num_devices=_NCORES)
        build_block(nc, cfg)
        _prefer_act_tables(nc.m.arch)
        nc.compile()
        self.nc = nc
        install_neuronx_cc_hook()

        in_names, out_names, out_avals, zero_outs = [], [], [], []
        pid = nc.partition_id_tensor.name if nc.partition_id_tensor else None
        self.pid_name = pid
        for alloc in nc.m.functions[0].allocations:
            if not isinstance(alloc, mybir.MemoryLocationSet):
                continue
            name = alloc.memorylocations[0].name
            if alloc.kind == "ExternalInput":
                if name != pid:
                    in_names.append(name)
            elif alloc.kind == "ExternalOutput":
                out_names.append(name)
                shape = tuple(alloc.tensor_shape)
                dtype = mybir.dt.np(alloc.dtype)
                out_avals.append(jax.core.ShapedArray(shape, dtype))
                zero_outs.append(np.zeros(shape, dtype))
        assert out_names == ["outT"]
        self.in_names = in_names
        self.out_names = out_names
        self.out_avals = out_avals
        self.zero_outs = zero_outs
        self.ix_xT = in_names.index("xT")
        self.fns = {}

    def fn(self, n_iters):
        if n_iters in self.fns:
            return self.fns[n_iters]
        n_params = len(self.in_names)
        all_in = tuple(self.in_names + self.out_names
                       + ([self.pid_name] if self.pid_name else []))
        pid = self.pid_name
        out_avals = tuple(self.out_avals)
        out_names = tuple(self.out_names)
        nc = self.nc
        ix = self.ix_xT

        def _call(x, ins, zouts):
            operands = ins[:ix] + [x] + ins[ix + 1:] + zouts
            if pid:
                operands = operands + [partition_id_tensor()]
            outs = _bass_exec_p.bind(
                *operands, out_avals=out_avals, in_names=all_in,
                out_names=out_names, lowering_input_output_aliases=(),
                sim_require_finite=True, sim_require_nnan=True, nc=nc)
            return outs[0]

        def _body(*args):
            ins = list(args[:n_params])
            zouts = list(args[n_params:])
            x = ins[ix]
            if n_iters == 1:
                return (_call(x, ins, zouts),)
            # neuronx_cc_hook allows one bass_exec per XLA module: use scan
            import jax.lax as lax

            def step(carry, _):
                return _call(carry, ins, zouts), None

            x, _ = lax.scan(step, x, None, length=n_iters)
            return (x,)

        mesh = Mesh(np.asarray(jax.devices()[:_NCORES]), ("core",))
        specs = (PartitionSpec("core"),) * (n_params + 1)
        f = jax.jit(shard_map(_body, mesh=mesh, in_specs=specs,
                              out_specs=(PartitionSpec("core"),),
                              check_rep=False))
        self.fns[n_iters] = f
        return f

    def concat_inputs(self, inputs):
        x = np.asarray(inputs["x"], np.float32)
        w1 = np.asarray(inputs["fc1_w"], np.float32)
        w18 = w1.astype(_F8)
        w1r8 = ((w1 - w18.astype(np.float32)) * RES_K).astype(_F8)
        shared = {
            "wqkv": np.ascontiguousarray(
                np.asarray(inputs["qkv_w"], np.float32)).astype(_F8),
            "wproj": np.asarray(inputs["proj_w"], np.float32).astype(_F8),
            "wfc1": w18,
            "wfc1r": w1r8,
            "wfc2": np.asarray(inputs["fc2_w"], np.float32).astype(_BF),
        }
        cfg = self.cfg
        for flag, names in (
            (cfg.apply_ln1_gb, ("ln1_g", "ln1_b")),
            (cfg.apply_ln2_gb, ("ln2_g", "ln2_b")),
            (cfg.apply_qkv_bias, ("qkv_b",)),
            (cfg.apply_proj_bias, ("proj_b",)),
            (cfg.apply_fc1_bias, ("fc1_b",)),
            (cfg.apply_fc2_bias, ("fc2_b",)),
        ):
            if flag:
                for n in names:
                    shared[n] = np.asarray(inputs[n], np.float32)
        per_core = []
        for c in range(_NCORES):
            b, q = divmod(c, _GROUP)
            xc = x[b, q * _T : (q + 1) * _T, :]
            m = {"xT": np.ascontiguousarray(xc.T), **shared}
            per_core.append([m[nm] for nm in self.in_names])
        concat = [np.concatenate([per_core[c][i] for c in range(_NCORES)], axis=0)
                  for i in range(len(self.in_names))]
        concat += [np.zeros((_NCORES * z.shape[0], *z.shape[1:]), z.dtype)
                   for z in self.zero_outs]
        return concat

    def run(self, inputs, n_iters=1):
        args = self.concat_inputs(inputs)
        out = self.fn(n_iters)(*args)
        jax.block_until_ready(out)
        return np.asarray(out[0])

    def measure_ns(self, inputs, n=50, reps=3):
        """Queued-chain wall estimate: upper bound incl. per-dispatch RPC."""
        args = self.concat_inputs(inputs)
        f = self.fn(1)
        lowered = f.lower(*args)
        compiled = lowered.compile()
        shardings = compiled.input_shardings[0]
        dev = [jax.device_put(a, s) for a, s in zip(args, shardings)]
        jax.block_until_ready(dev)
        ix = self.ix_xT
        out = f(*dev)
        jax.block_until_ready(out)
        best = None
        for _ in range(reps):
            x = dev[ix]
            t0 = _time.perf_counter()
            for _i in range(n):
                out = f(*(dev[:ix] + [x] + dev[ix + 1:]))
                x = out[0]
            jax.block_until_ready(out)
            est = (_time.perf_counter() - t0) / n
            best = est if best is None else min(best, est)
        return best * 1e9


def _get_runner(inputs):
    cfg = _build_cfg(inputs)
    key = (cfg.apply_ln1_gb, cfg.apply_ln2_gb, cfg.apply_qkv_bias,
           cfg.apply_proj_bias, cfg.apply_fc1_bias, cfg.apply_fc2_bias)
    if key not in _CACHE:
        _CACHE[key] = _Runner(cfg)
    return _CACHE[key]


def kernel(**inputs) -> np.ndarray:
    r = _get_runner(inputs)
    flat = r.run(inputs)  # [8*DIM, T] stacked per-core outT
    out = np.empty((_B, _N, _DIM), np.float32)
    for c in range(_NCORES):
        b, q = divmod(c, _GROUP)
        out[b, q * _T : (q + 1) * _T, :] = flat[c * _DIM : (c + 1) * _DIM, :].T
    return out


def measure_hw_time_ns(**inputs) -> float:
    """Estimate per-execution device time by differencing chained runs."""
    return _get_runner(inputs).measure_ns(inputs)


# revision 52
# speedup vs baseline: 1.3927x; 1.1001x over previous
"""nn_Block dense_transformer kernel for 8 TRN2 NeuronCores.

Self-contained: builds the Bass/Tile program, shards inputs across 8 cores
(sequence-parallel, 4 cores per batch element), runs via a cached jitted
shard_map over jax's neuron devices, gathers the full output.

Numerics: fp8-e4m3 DoubleRow matmuls for qkv / QK / AV / proj / fc1 (fc1 with
scaled-residual error correction), bf16 fc2, exp->fp8 softmax with a
softmax-invariant score shift; part of the exp work runs on DVE via a
Schraudolph-style uint8 bit trick. Validated ~5.7e-3 rel err vs f32 ref.
"""
import os
import numpy as np
import ml_dtypes


from contextlib import ExitStack
from dataclasses import dataclass

import concourse.bass as bass
import concourse.mybir as mybir
import concourse.tile as tile

F32 = mybir.dt.float32
F32R = mybir.dt.float32r
BF16 = mybir.dt.bfloat16
F8 = mybir.dt.float8e4
U8 = mybir.dt.uint8
P = 128
ALU = mybir.AluOpType
ACTF = mybir.ActivationFunctionType
DR = mybir.MatmulPerfMode.DoubleRow

LOG2E = 1.4426950408889634
RES_K = 32.0   # fc1 residual pre-scale


@dataclass
class Cfg:
    dim: int = 1024
    heads: int = 16
    hd: int = 64
    hidden: int = 4096
    T: int = 512          # tokens per core
    group: int = 4        # cores per batch group
    n_cores: int = 8
    eps: float = 1e-5
    shift: float = 2.6    # softmax-invariant score shift (fit e^s into e4m3)
    # build-time specialization flags (host inspects actual input values)
    apply_ln1_gb: bool = False
    apply_ln2_gb: bool = False
    apply_qkv_bias: bool = False
    apply_proj_bias: bool = False
    apply_fc1_bias: bool = False
    apply_fc2_bias: bool = False
    fake_ag: bool = False  # timing-only: skip collective, read own kv as all ranks
    # j-indices (within each pair's 16 j-steps) whose exp runs on DVE via the
    # Schraudolph uint8 trick; the rest run on ACT (native exp)
    dve_js: tuple = (1, 3, 6, 9, 12, 14)
    schr_round: float = 0.0  # probe-verified: DVE f32->u8 rounds-to-nearest
    stop_after: str = "full"  # bisect: qkv|ag|kvload|attn|proj|fc1|full

    @property
    def dch(self):
        return self.dim // P

    @property
    def hch(self):
        return self.hidden // P

    @property
    def kt(self):
        return (self.group * self.T) // P

    @property
    def tpr(self):
        return self.T // P

    @property
    def pairs(self):
        return self.heads // 2

    @property
    def vf(self):  # free-dim chunk for the v matmul
        return min(512, self.dim)

    @property
    def schr_b(self):
        # uint8(LOG2E * raw_score + schr_b) bitcast e4m3 ~= exp(score/8 - shift)
        return 56.0 - 8.0 * LOG2E * self.shift - 8.0 * 0.043677 + self.schr_round


def build_block(nc: bass.Bass, cfg: Cfg):
    dch, hch, kt, T, hd = cfg.dch, cfg.hch, cfg.kt, cfg.T, cfg.hd

    xT = nc.dram_tensor("xT", [cfg.dim, T], F32, kind="ExternalInput").ap()
    wqkv = nc.dram_tensor("wqkv", [cfg.dim, 3 * cfg.dim], F8, kind="ExternalInput").ap()
    wproj = nc.dram_tensor("wproj", [cfg.dim, cfg.dim], F8, kind="ExternalInput").ap()
    wfc1 = nc.dram_tensor("wfc1", [cfg.dim, cfg.hidden], F8, kind="ExternalInput").ap()
    wfc1r = nc.dram_tensor("wfc1r", [cfg.dim, cfg.hidden], F8, kind="ExternalInput").ap()
    wfc2 = nc.dram_tensor("wfc2", [cfg.hidden, cfg.dim], F8, kind="ExternalInput").ap()
    wfc2r = nc.dram_tensor("wfc2r", [cfg.hidden, cfg.dim], F8, kind="ExternalInput").ap()
    outT = nc.dram_tensor("outT", [cfg.dim, T], F32, kind="ExternalOutput").ap()

    ln1_g = ln1_b = ln2_g = ln2_b = None
    if cfg.apply_ln1_gb:
        ln1_g = nc.dram_tensor("ln1_g", [cfg.dim], F32, kind="ExternalInput").ap()
        ln1_b = nc.dram_tensor("ln1_b", [cfg.dim], F32, kind="ExternalInput").ap()
    if cfg.apply_ln2_gb:
        ln2_g = nc.dram_tensor("ln2_g", [cfg.dim], F32, kind="ExternalInput").ap()
        ln2_b = nc.dram_tensor("ln2_b", [cfg.dim], F32, kind="ExternalInput").ap()
    qkv_b = proj_b = fc1_b = fc2_b = None
    if cfg.apply_qkv_bias:
        qkv_b = nc.dram_tensor("qkv_b", [3 * cfg.dim], F32, kind="ExternalInput").ap()
    if cfg.apply_proj_bias:
        proj_b = nc.dram_tensor("proj_b", [cfg.dim], F32, kind="ExternalInput").ap()
    if cfg.apply_fc1_bias:
        fc1_b = nc.dram_tensor("fc1_b", [cfg.hidden], F32, kind="ExternalInput").ap()
    if cfg.apply_fc2_bias:
        fc2_b = nc.dram_tensor("fc2_b", [cfg.dim], F32, kind="ExternalInput").ap()

    # collective bounce buffers, fp8 (k first so its AllGather can complete and
    # feed QK while v's AllGather still runs)
    half = cfg.dim * T
    cck_in = nc.dram_tensor("cck_in", [half], F8, kind="Internal").ap()
    cck_out = nc.dram_tensor("cck_out", [cfg.group * half], F8, kind="Internal").ap()
    ccv_in = nc.dram_tensor("ccv_in", [half], F8, kind="Internal").ap()
    ccv_out = nc.dram_tensor("ccv_out", [cfg.group * half], F8, kind="Internal").ap()

    def dram_view(ap, off, shape_strides):
        return bass.AP(tensor=ap.tensor, offset=ap.offset + off,
                       ap=[[s, n] for s, n in shape_strides])

    LEVELS = ["qkv", "ag", "kvload", "attn", "proj", "ln2", "fc1", "full"]
    _stop = LEVELS.index(cfg.stop_after)
    go = lambda ph: _stop >= LEVELS.index(ph)  # noqa: E731

    with tile.TileContext(nc) as tc, ExitStack() as ctx:
        const = ctx.enter_context(tc.tile_pool(name="const", bufs=1))
        rows = ctx.enter_context(tc.tile_pool(name="rows", bufs=2))
        bcast = ctx.enter_context(tc.tile_pool(name="bcast", bufs=2))
        evict = ctx.enter_context(tc.tile_pool(name="evict", bufs=3))
        tmp = ctx.enter_context(tc.tile_pool(name="tmp", bufs=2))
        # pool release is strict LIFO: open long-lived pools first
        x2p = ctx.enter_context(tc.tile_pool(name="x2p", bufs=1))
        es_qr = ExitStack()     # qr: lives until end of attention
        es_proj = ExitStack()   # freed after proj: xT, oT, wproj
        es_q = ExitStack()      # qT_8: freed after the qr remap
        wfc1p = ctx.enter_context(tc.tile_pool(name="wfc1", bufs=1))
        qrp = es_qr.enter_context(tc.tile_pool(name="qrp", bufs=1))
        xpool = es_proj.enter_context(tc.tile_pool(name="xpool", bufs=1))
        opool = es_proj.enter_context(tc.tile_pool(name="opool", bufs=1))
        wprojp = es_proj.enter_context(tc.tile_pool(name="wproj", bufs=4))
        qpool = es_q.enter_context(tc.tile_pool(name="qpool", bufs=1))

        ones_bf = const.tile([P, 1], BF16, tag="onesbf", bufs=1)
        nc.vector.memset(ones_bf, 1.0)
        eps_t = const.tile([1, 1], F32, tag="eps", bufs=1)
        nc.vector.memset(eps_t, cfg.eps)
        nshift_t = const.tile([P, 1], F32, tag="nshift", bufs=1)
        nc.vector.memset(nshift_t, -cfg.shift)

        xT_sb = xpool.tile([P, dch, T], F32, name="xT_sb", tag="xT", bufs=1)
        for c in range(0, dch, 2):
            nc.sync.dma_start(
                out=xT_sb[:, c : c + 2, :],
                in_=dram_view(xT, c * P * T, [(T, P), (P * T, 2), (1, T)]))

        def load_cols(vec, nch, nm):
            t = const.tile([P, nch], F32, name=nm, tag=nm, bufs=1)
            nc.sync.dma_start(out=t, in_=vec.rearrange("(c p) -> p c", p=P))
            return t

        ln1g_sb = load_cols(ln1_g, dch, "ln1g") if ln1_g is not None else None
        ln1b_sb = load_cols(ln1_b, dch, "ln1b") if ln1_b is not None else None
        ln2g_sb = load_cols(ln2_g, dch, "ln2g") if ln2_g is not None else None
        ln2b_sb = load_cols(ln2_b, dch, "ln2b") if ln2_b is not None else None
        qkvb_sb = load_cols(qkv_b, 3 * dch, "qkvb") if qkv_b is not None else None
        projb_sb = load_cols(proj_b, dch, "projb") if proj_b is not None else None
        fc1b_sb = load_cols(fc1_b, hch, "fc1b") if fc1_b is not None else None
        fc2b_sb = load_cols(fc2_b, dch, "fc2b") if fc2_b is not None else None

        # ---- LayerNorm stats over features (feature-major data) ----
        # returns (r1b, r2b) broadcast rows: normalized = x*r1b + r2b
        def ln_stats(psln, x_sb):
            sum_x = psln.tile([1, T], F32, name="sum_x", tag="sumx", bufs=1)
            sum_sq = psln.tile([1, T], F32, name="sum_sq", tag="sumsq", bufs=1)
            for c in range(dch):
                eng = nc.gpsimd if c % 2 else nc.vector
                eng2 = nc.vector if c % 2 else nc.gpsimd
                xb = tmp.tile([P, T], BF16, name="xb", tag="xb")
                eng.tensor_copy(out=xb, in_=x_sb[:, c, :])
                xsq = tmp.tile([P, T], BF16, name="xsq", tag="xsq")
                eng2.tensor_mul(xsq, x_sb[:, c, :], x_sb[:, c, :])
                nc.tensor.matmul(sum_x, lhsT=ones_bf, rhs=xb,
                                 start=(c == 0), stop=(c == dch - 1))
                nc.tensor.matmul(sum_sq, lhsT=ones_bf, rhs=xsq,
                                 start=(c == 0), stop=(c == dch - 1))
            inv_d = 1.0 / cfg.dim
            m_row = rows.tile([1, T], F32, name="m_row", bufs=1)
            var_row = rows.tile([1, T], F32, name="var_row", bufs=1)
            nc.vector.tensor_scalar(out=m_row, in0=sum_x, scalar1=inv_d,
                                    scalar2=None, op0=ALU.mult)
            nc.vector.tensor_mul(var_row, m_row, m_row)           # m^2
            nc.vector.scalar_tensor_tensor(                        # sum_sq/D - m^2
                out=var_row, in0=sum_sq, scalar=inv_d, in1=var_row,
                op0=ALU.mult, op1=ALU.subtract)
            lnv = rows.tile([1, T], F32, name="lnv", bufs=1)
            rstd = rows.tile([1, T], F32, name="rstd", bufs=1)
            nc.scalar.activation(lnv, var_row, ACTF.Ln, bias=eps_t)
            nc.scalar.activation(rstd, lnv, ACTF.Exp, scale=-0.5)
            r2 = rows.tile([1, T], F32, name="r2", bufs=1)         # -m * rstd
            nc.vector.scalar_tensor_tensor(
                out=r2, in0=m_row, scalar=-1.0, in1=rstd,
                op0=ALU.mult, op1=ALU.mult)
            r1b = bcast.tile([P, T], F32, name="r1b", bufs=1)
            r2b = bcast.tile([P, T], F32, name="r2b", bufs=1)
            nc.gpsimd.partition_broadcast(r1b, rstd)
            nc.gpsimd.partition_broadcast(r2b, r2)
            return r1b, r2b

        # ---------------- Phase 1: LN1 + QKV(fp8 DR) + AllGather ----------------
        qT_8 = qpool.tile([P, dch, T], F8, name="qT_8")
        with tc.tile_pool(name="h1", bufs=1) as h1p, \
             tc.tile_pool(name="wqkv", bufs=4) as wqkvp, \
             tc.tile_pool(name="psln1", bufs=1, space="PSUM") as psln1, \
             tc.tile_pool(name="psqkv", bufs=4, space="PSUM") as psq:
            h1_8 = h1p.tile([P, dch, T], F8, name="h1_8")
            r1b, r2b = ln_stats(psln1, xT_sb)
            for c in range(dch):
                eng = nc.gpsimd if c % 2 else nc.vector
                t0 = tmp.tile([P, T], F32, name="ln_t0", tag="t0")
                eng.tensor_mul(t0, xT_sb[:, c, :], r1b)
                if ln1g_sb is not None:
                    t1 = tmp.tile([P, T], F32, name="ln_t1", tag="t1")
                    eng.tensor_add(t1, t0, r2b)
                    eng.tensor_scalar(
                        out=h1_8[:, c, :], in0=t1,
                        scalar1=ln1g_sb[:, c : c + 1], scalar2=ln1b_sb[:, c : c + 1],
                        op0=ALU.mult, op1=ALU.add)
                else:
                    eng.tensor_add(h1_8[:, c, :], t0, r2b)

            wq = []
            for u in range(dch // 2):
                w = wqkvp.tile([P, 2, 3 * cfg.dim], F8, name="wqkv_t")
                nc.sync.dma_start(
                    out=w, in_=dram_view(
                        wqkv, 2 * u * P * 3 * cfg.dim,
                        [(3 * cfg.dim, P), (P * 3 * cfg.dim, 2), (1, 3 * cfg.dim)]))
                wq.append(w)

            def qkv_evict(ps, dst, col_idx):
                # on ACT: it is idle during this phase while DVE is loaded
                if qkvb_sb is not None:
                    nc.scalar.activation(dst, ps, ACTF.Identity,
                                         bias=qkvb_sb[:, col_idx : col_idx + 1])
                else:
                    nc.scalar.copy(dst, ps)

            # k first (feeds the AllGather + kr loads), then q (gates the qr
            # remap and so the attention start), then v (only needed at the
            # first AV, a few us into attention). k/q evict on ACT, v on DVE.
            kfull = evict.tile([P, dch, T], F8, name="kfull", tag="kfull", bufs=1)
            for m in range(dch):  # k, feature-major, fp8
                ps = psq.tile([P, T], F32, name="ps_qk", tag="ps_qkv")
                for u in range(dch // 2):
                    nc.tensor.matmul(
                        ps, lhsT=wq[u][:, :, (dch + m) * P : (dch + m + 1) * P],
                        rhs=h1_8[:, 2 * u : 2 * u + 2, :],
                        start=(u == 0), stop=(u == dch // 2 - 1), perf_mode=DR)
                qkv_evict(ps, kfull[:, m, :], dch + m)
            nc.sync.dma_start(
                out=dram_view(cck_in, 0, [(T, P), (P * T, dch), (1, T)]),
                in_=kfull)
            for m in range(dch):  # q, feature-major (stays local), fp8
                ps = psq.tile([P, T], F32, name="ps_q", tag="ps_qkv")
                for u in range(dch // 2):
                    nc.tensor.matmul(
                        ps, lhsT=wq[u][:, :, m * P : (m + 1) * P],
                        rhs=h1_8[:, 2 * u : 2 * u + 2, :],
                        start=(u == 0), stop=(u == dch // 2 - 1), perf_mode=DR)
                qkv_evict(ps, qT_8[:, m, :], m)
            vfull = evict.tile([P, cfg.tpr, 2, cfg.vf], F8, name="vfull",
                               tag="vfull", bufs=1)
            for tm in range(cfg.tpr):  # v, token-major, fp8
                for f in range(cfg.dim // cfg.vf):
                    ps = psq.tile([P, cfg.vf], F32, name="ps_v", tag="ps_qkv")
                    for u in range(dch // 2):
                        nc.tensor.matmul(
                            ps, lhsT=h1_8[:, 2 * u : 2 * u + 2, tm * P : (tm + 1) * P],
                            rhs=wq[u][:, :, 2 * cfg.dim + f * cfg.vf
                                      : 2 * cfg.dim + (f + 1) * cfg.vf],
                            start=(u == 0), stop=(u == dch // 2 - 1), perf_mode=DR)
                    # v bias folded in after the softmax (rows sum to 1)
                    nc.vector.tensor_copy(out=vfull[:, tm, f, :], in_=ps)
            nc.sync.dma_start(
                out=dram_view(ccv_in, 0,
                              [(cfg.dim, P), (P * cfg.dim, cfg.tpr), (1, cfg.dim)]),
                in_=vfull)

        if go("ag") and not cfg.fake_ag:
            groups = [list(range(g * cfg.group, (g + 1) * cfg.group))
                      for g in range(cfg.n_cores // cfg.group)]
            nc.gpsimd.collective_compute(
                "AllGather", ALU.bypass,
                ins=[cck_in], outs=[cck_out], replica_groups=groups)
            nc.gpsimd.collective_compute(
                "AllGather", ALU.bypass,
                ins=[ccv_in], outs=[ccv_out], replica_groups=groups)

        # proj + fc1 weight tiles (fp8, c-pair layout for DoubleRow); allocated
        # now, but DMA issue is deferred into the attention loop so the
        # attention-critical kr/v loads hit the DMA path first
        wp = [wprojp.tile([P, 2, cfg.dim], F8, name="wproj_t")
              for _ in range(dch // 2)]
        w1m = [wfc1p.tile([P, 2, cfg.hidden], F8, name="wfc1_m", tag=f"w1m{u}",
                          bufs=1) for u in range(dch // 2)]
        w1r = [wfc1p.tile([P, 2, cfg.hidden], F8, name="wfc1_r", tag=f"w1r{u}",
                          bufs=1) for u in range(dch // 2)]

        def pair_view(nrow):
            return [(nrow, P), (P * nrow, 2), (1, nrow)]

        _deferred = []
        for u in range(dch // 2):
            _deferred.append((wp[u], dram_view(
                wproj, 2 * u * P * cfg.dim, pair_view(cfg.dim))))
        for u in range(dch // 2):
            _deferred.append((w1m[u], dram_view(
                wfc1, 2 * u * P * cfg.hidden, pair_view(cfg.hidden))))
            _deferred.append((w1r[u], dram_view(
                wfc1r, 2 * u * P * cfg.hidden, pair_view(cfg.hidden))))

        # ---------------- Phase 2: attention ----------------
        # q remap to DoubleRow layout [32, half, head, T] via 4 SBUF DMAs
        qr = qrp.tile([32, 2, cfg.heads, T], F8, name="qr")
        for hp in range(2):
            for hf in range(2):
                p0 = hp * 64 + hf * 32
                nc.sync.dma_start(
                    out=qr[:, hf, hp : cfg.heads : 2, :],
                    in_=qT_8[p0 : p0 + 32, :, :])
        es_q.close()  # qT_8 freed

        oT_8 = opool.tile([P, dch, T], F8, name="oT_8")
        with tc.tile_pool(name="krp", bufs=2) as krp, \
             tc.tile_pool(name="vaugp", bufs=1) as vaugp, \
             tc.tile_pool(name="expp", bufs=3) as expp, \
             tc.tile_pool(name="psS", bufs=3, space="PSUM") as psS, \
             tc.tile_pool(name="psO", bufs=1, space="PSUM") as psO:
            def load_kr(p):
                # k for pair p, DoubleRow layout [32, head(2), half, keys];
                # (head, half) merge on both sides -> one DMA per rank
                kr = krp.tile([32, 2, 2, cfg.group * T], F8, name="kr", tag="kr")
                for r in range(cfg.group):
                    ksrc = [(T, 32), (32 * T, 4), (1, T)]
                    koff = 2 * p * hd * T
                    nc.sync.dma_start(
                        out=kr[:, :, :, r * T : (r + 1) * T],
                        in_=dram_view(cck_in, koff, ksrc) if cfg.fake_ag
                        else dram_view(cck_out, r * half + koff, ksrc))
                return kr

            v_aug = vaugp.tile([P, kt, cfg.heads, hd + 1], F8, name="v_aug")
            nc.vector.memset(v_aug[:, :, :, hd : hd + 1], 1.0)
            kr_next = load_kr(0) if go("attn") else None  # before the v stream
            for r in range(cfg.group if go("kvload") else 0):
                for jt in range(cfg.tpr):
                    vsrc = [(cfg.dim, P), (hd, cfg.heads), (1, hd)]
                    voff = jt * P * cfg.dim
                    nc.sync.dma_start(
                        out=v_aug[:, r * cfg.tpr + jt, :, :hd],
                        in_=dram_view(ccv_in, voff, vsrc) if cfg.fake_ag
                        else dram_view(ccv_out, r * half + voff, vsrc))

            x2_sb = x2p.tile([P, dch, T], F32, name="x2_sb", tag="x2", bufs=1)
            for p in range(cfg.pairs if go("attn") else 0):
                kr = kr_next
                if p + 1 < cfg.pairs:
                    kr_next = load_kr(p + 1)
                # drip the deferred proj/fc1 weight loads into the late
                # (DMA-idle) half of the attention window
                if p >= 4:
                    for w, src in _deferred[4 * (p - 4) : 4 * (p - 3)]:
                        nc.sync.dma_start(out=w, in_=src)

                psO_A = psO.tile([hd + 1, T], F32, name="psO_A", tag="oA")
                psO_B = psO.tile([hd + 1, T], F32, name="psO_B", tag="oB")
                for uj in range(kt // 2):
                    es = expp.tile([P, 2, 2, T], F8, name="es", tag="es")
                    es_u8 = es.bitcast(U8)
                    for i in range(2):
                        j = 2 * uj + i
                        s = psS.tile([P, 2, T], F32, name="s_qk", tag="s")
                        kcol = slice(j * P, (j + 1) * P)
                        for hh in range(2):
                            nc.tensor.matmul(
                                s[:, hh, :], lhsT=kr[:, hh, :, kcol],
                                rhs=qr[:, :, 2 * p + hh, :],
                                start=True, stop=True, perf_mode=DR)
                        if j in cfg.dve_js or (p % 2 and j == 15):
                            # Schraudolph exp on DVE: uint8(A*raw+B) bits = e4m3
                            nc.vector.tensor_scalar(
                                out=es_u8[:, i, :, :], in0=s,
                                scalar1=LOG2E, scalar2=cfg.schr_b,
                                op0=ALU.mult, op1=ALU.add)
                        else:
                            nc.scalar.activation(
                                es[:, i, :, :], s, ACTF.Exp,
                                scale=0.125, bias=nshift_t)
                    nc.tensor.matmul(
                        psO_A, lhsT=v_aug[:, 2 * uj : 2 * uj + 2, 2 * p, :],
                        rhs=es[:, :, 0, :],
                        start=(uj == 0), stop=(uj == kt // 2 - 1), perf_mode=DR)
                    nc.tensor.matmul(
                        psO_B, lhsT=v_aug[:, 2 * uj : 2 * uj + 2, 2 * p + 1, :],
                        rhs=es[:, :, 1, :],
                        start=(uj == 0), stop=(uj == kt // 2 - 1), perf_mode=DR)
                for h, pso, poff in ((2 * p, psO_A, 0), (2 * p + 1, psO_B, 64)):
                    rrow = rows.tile([1, T], F32, name="rrow")
                    nc.vector.reciprocal(rrow, pso[hd : hd + 1, :])
                    rb = bcast.tile([P, T], F32, name="rb")
                    nc.gpsimd.partition_broadcast(rb[0:hd, :], rrow)
                    dst = oT_8[poff : poff + hd, p, :]
                    nc.vector.tensor_mul(dst, pso[0:hd, :], rb[0:hd, :])
                    if qkvb_sb is not None:  # v bias: softmax rows sum to 1
                        nc.vector.tensor_scalar_add(
                            out=dst, in0=dst,
                            scalar1=qkvb_sb[poff : poff + hd,
                                            2 * dch + p : 2 * dch + p + 1])

        # ---------------- proj (fp8 DR, PSUM-accumulated) + residual ----------
        if go("proj"):
            with tc.tile_pool(name="pspp", bufs=2, space="PSUM") as pspp:
                for m in range(dch):
                    pp = pspp.tile([P, T], F32, name="ps_pp")
                    for u in range(dch // 2):
                        nc.tensor.matmul(
                            pp, lhsT=wp[u][:, :, m * P : (m + 1) * P],
                            rhs=oT_8[:, 2 * u : 2 * u + 2, :],
                            start=(u == 0), stop=(u == dch // 2 - 1), perf_mode=DR)
                    if projb_sb is not None:
                        nc.vector.tensor_scalar_add(
                            out=pp, in0=pp, scalar1=projb_sb[:, m : m + 1])
                    nc.vector.tensor_add(x2_sb[:, m, :], pp, xT_sb[:, m, :])

        es_proj.close()  # xT, oT, wproj no longer needed
        es_qr.close()    # qr freed

        # ------------- Phase 3: LN2 + fc1(fp8 DR, corrected) + gelu + fc2 -----
        gpool = ctx.enter_context(tc.tile_pool(name="gpool", bufs=1))
        g8_sb = gpool.tile([P, hch, T], F8, name="g8_sb")
        g8d_sb = gpool.tile([P, hch, T], F8, name="g8d_sb")
        l28_sb = gpool.tile([P, hch, T], F8, name="l28_sb")
        with tc.tile_pool(name="h2", bufs=1) as h2p:
            h2_8 = h2p.tile([P, dch, T], F8, name="h2_8")
            h2d_8 = h2p.tile([P, dch, T], F8, name="h2d_8")
            l2_8 = h2p.tile([P, dch, T], F8, name="l2_8")
            with tc.tile_pool(name="psln2", bufs=1, space="PSUM") as psln2:
                if go("ln2"):
                    r1b, r2b = ln_stats(psln2, x2_sb)
                    for c in range(dch):
                        eng = nc.gpsimd if c % 2 else nc.vector
                        t0 = tmp.tile([P, T], F32, name="l2t0", tag="t0")
                        t1 = tmp.tile([P, T], F32, name="l2t1", tag="t1")
                        eng.tensor_mul(t0, x2_sb[:, c, :], r1b)
                        if ln2g_sb is not None:
                            eng.tensor_add(t1, t0, r2b)
                            eng.tensor_scalar(
                                out=t1, in0=t1,
                                scalar1=ln2g_sb[:, c : c + 1],
                                scalar2=ln2b_sb[:, c : c + 1],
                                op0=ALU.mult, op1=ALU.add)
                        else:
                            eng.tensor_add(t1, t0, r2b)
                        # fc1 correction operands, pre-scaled so all three
                        # matmul series accumulate into ONE psum bank:
                        #   h8*W8 + fp8(t1/K)*RW8 + fp8(t1-h8)*W8
                        nc.scalar.copy(h2_8[:, c, :], t1)
                        nc.scalar.mul(h2d_8[:, c, :], t1, 1.0 / RES_K)
                        nc.vector.tensor_tensor(
                            out=l2_8[:, c, :], in0=t1, in1=h2_8[:, c, :],
                            op=ALU.subtract)

            with tc.tile_pool(name="psm", bufs=4, space="PSUM") as psm:
                for m in range(hch if go("fc1") else 0):
                    ps = psm.tile([P, T], F32, name="ps_fc1", tag="fm")
                    for u in range(dch // 2):
                        nc.tensor.matmul(
                            ps, lhsT=w1m[u][:, :, m * P : (m + 1) * P],
                            rhs=h2_8[:, 2 * u : 2 * u + 2, :],
                            start=(u == 0), stop=False, perf_mode=DR)
                    for u in range(dch // 2):
                        nc.tensor.matmul(
                            ps, lhsT=w1r[u][:, :, m * P : (m + 1) * P],
                            rhs=h2d_8[:, 2 * u : 2 * u + 2, :],
                            start=False, stop=False, perf_mode=DR)
                    for u in range(dch // 2):
                        nc.tensor.matmul(
                            ps, lhsT=w1m[u][:, :, m * P : (m + 1) * P],
                            rhs=l2_8[:, 2 * u : 2 * u + 2, :],
                            start=False, stop=(u == dch // 2 - 1), perf_mode=DR)
                    # gelu in bf16, then fp8 main + W/X correction operands
                    # (same single-psum-accumulation trick as fc1)
                    gbf = tmp.tile([P, T], BF16, name="gbf", tag="gbf")
                    nc.scalar.activation(
                        gbf, ps, ACTF.Gelu,
                        bias=fc1b_sb[:, m : m + 1] if fc1b_sb is not None else 0.0)
                    nc.scalar.copy(g8_sb[:, m, :], gbf)
                    nc.vector.tensor_tensor(
                        out=l28_sb[:, m, :], in0=gbf, in1=g8_sb[:, m, :],
                        op=ALU.subtract)
                    eng = nc.gpsimd if m % 2 else nc.vector
                    eng.tensor_scalar(
                        out=g8d_sb[:, m, :], in0=gbf, scalar1=1.0 / RES_K,
                        scalar2=None, op0=ALU.mult)

        with tc.tile_pool(name="wfc2", bufs=4) as wfc2p, \
             tc.tile_pool(name="psf2", bufs=1, space="PSUM") as psf2:
            acc = [psf2.tile([P, T], F32, name=f"ps_fc2_{m}", tag=f"acc{m}",
                             bufs=1) for m in range(dch)]
            ncu = hch // 2
            for cu in range(ncu if go("full") else 0):
                w2 = wfc2p.tile([P, 2, cfg.dim], F8, name="wfc2_t", tag="w2m")
                w2r = wfc2p.tile([P, 2, cfg.dim], F8, name="wfc2r_t", tag="w2r")
                nc.sync.dma_start(out=w2, in_=dram_view(
                    wfc2, 2 * cu * P * cfg.dim, pair_view(cfg.dim)))
                nc.sync.dma_start(out=w2r, in_=dram_view(
                    wfc2r, 2 * cu * P * cfg.dim, pair_view(cfg.dim)))
                for m in range(dch):
                    lhs = slice(m * P, (m + 1) * P)
                    cs = slice(2 * cu, 2 * cu + 2)
                    nc.tensor.matmul(
                        acc[m], lhsT=w2[:, :, lhs], rhs=g8_sb[:, cs, :],
                        start=(cu == 0), stop=False, perf_mode=DR)
                    nc.tensor.matmul(
                        acc[m], lhsT=w2r[:, :, lhs], rhs=g8d_sb[:, cs, :],
                        start=False, stop=False, perf_mode=DR)
                    nc.tensor.matmul(
                        acc[m], lhsT=w2[:, :, lhs], rhs=l28_sb[:, cs, :],
                        start=False, stop=(cu == ncu - 1), perf_mode=DR)
            # final residual add in place into x2, then store in two DMAs
            for m in range(dch if go("full") else 0):
                if fc2b_sb is not None:
                    nc.vector.tensor_scalar_add(
                        out=acc[m], in0=acc[m], scalar1=fc2b_sb[:, m : m + 1])
                nc.vector.tensor_add(x2_sb[:, m, :], acc[m], x2_sb[:, m, :])
                if go("full") and m % (dch // 2) == dch // 2 - 1:
                    m0 = m - dch // 2 + 1
                    nc.sync.dma_start(
                        out=dram_view(outT, m0 * P * T,
                                      [(T, P), (P * T, dch // 2), (1, T)]),
                        in_=x2_sb[:, m0 : m + 1, :])

    return nc


# ----------------------------------------------------------------------------
# host wrapper
# ----------------------------------------------------------------------------
import time as _time

import jax
from jax.sharding import Mesh, PartitionSpec
from jax.experimental.shard_map import shard_map

from concourse import bacc
from concourse.bass2jax import (_bass_exec_p, install_neuronx_cc_hook,
                                partition_id_tensor)

_BF = ml_dtypes.bfloat16
_F8 = ml_dtypes.float8_e4m3
_DIM, _HEADS, _HIDDEN = 1024, 16, 4096
_B, _N = 2, 2048
_GROUP, _NCORES = 4, 8
_T = _B * _N // _NCORES

_CACHE = {}


def _build_cfg(inputs):
    def nz(a):
        return bool(np.any(np.asarray(a)))

    return Cfg(
        dim=_DIM, heads=_HEADS, hidden=_HIDDEN, T=_T, group=_GROUP,
        n_cores=_NCORES,
        apply_ln1_gb=not (np.allclose(inputs["ln1_g"], 1.0)
                          and not nz(inputs["ln1_b"])),
        apply_ln2_gb=not (np.allclose(inputs["ln2_g"], 1.0)
                          and not nz(inputs["ln2_b"])),
        apply_qkv_bias=nz(inputs["qkv_b"]),
        apply_proj_bias=nz(inputs["proj_b"]),
        apply_fc1_bias=nz(inputs["fc1_b"]),
        apply_fc2_bias=nz(inputs["fc2_b"]),
    )


def _prefer_act_tables(arch):
    """No-op: reordering the cached act-table dict desyncs act_func_set_id
    (a positional index into act_info.json) from the walrus-side mapping."""


class _Runner:
    def __init__(self, cfg):
        import concourse.mybir as mybir

        self.cfg = cfg
        nc = bacc.Bacc("TRN2", target_bir_lowering=False, debug=False,
                       num_devices=_NCORES)
        build_block(nc, cfg)
        _prefer_act_tables(nc.m.arch)
        nc.compile()
        self.nc = nc
        install_neuronx_cc_hook()

        in_names, out_names, out_avals, zero_outs = [], [], [], []
        pid = nc.partition_id_tensor.name if nc.partition_id_tensor else None
        self.pid_name = pid
        for alloc in nc.m.functions[0].allocations:
            if not isinstance(alloc, mybir.MemoryLocationSet):
                continue
            name = alloc.memorylocations[0].name
            if alloc.kind == "ExternalInput":
                if name != pid:
                    in_names.append(name)
            elif alloc.kind == "ExternalOutput":
                out_names.append(name)
                shape = tuple(alloc.tensor_shape)
                dtype = mybir.dt.np(alloc.dtype)
                out_avals.append(jax.core.ShapedArray(shape, dtype))
                zero_outs.append(np.zeros(shape, dtype))
        assert out_names == ["outT"]
        self.in_names = in_names
        self.out_names = out_names
        self.out_avals = out_avals
        self.zero_outs = zero_outs
        self.ix_xT = in_names.index("xT")
        self.fns = {}

    def fn(self, n_iters):
        if n_iters in self.fns:
            return self.fns[n_iters]
        n_params = len(self.in_names)
        all_in = tuple(self.in_names + self.out_names
                       + ([self.pid_name] if self.pid_name else []))
        pid = self.pid_name
        out_avals = tuple(self.out_avals)
        out_names = tuple(self.out_names)
        nc = self.nc
        ix = self.ix_xT

        def _call(x, ins, zouts):
            operands = ins[:ix] + [x] + ins[ix + 1:] + zouts
            if pid:
                operands = operands + [partition_id_tensor()]
            outs = _bass_exec_p.bind(
                *operands, out_avals=out_avals, in_names=all_in,
                out_names=out_names, lowering_input_output_aliases=(),
                sim_require_finite=True, sim_require_nnan=True, nc=nc)
            return outs[0]

        def _body(*args):
            ins = list(args[:n_params])
            zouts = list(args[n_params:])
            x = ins[ix]
            if n_iters == 1:
                return (_call(x, ins, zouts),)
            # neuronx_cc_hook allows one bass_exec per XLA module: use scan
            import jax.lax as lax

            def step(carry, _):
                return _call(carry, ins, zouts), None

            x, _ = lax.scan(step, x, None, length=n_iters)
            return (x,)

        mesh = Mesh(np.asarray(jax.devices()[:_NCORES]), ("core",))
        specs = (PartitionSpec("core"),) * (n_params + 1)
        f = jax.jit(shard_map(_body, mesh=mesh, in_specs=specs,
                              out_specs=(PartitionSpec("core"),),
                              check_rep=False))
        self.fns[n_iters] = f
        return f

    def concat_inputs(self, inputs):
        x = np.asarray(inputs["x"], np.float32)
        w1 = np.asarray(inputs["fc1_w"], np.float32)
        w18 = w1.astype(_F8)
        w2 = np.asarray(inputs["fc2_w"], np.float32)
        w28 = w2.astype(_F8)
        shared = {
            "wqkv": np.ascontiguousarray(
                np.asarray(inputs["qkv_w"], np.float32)).astype(_F8),
            "wproj": np.asarray(inputs["proj_w"], np.float32).astype(_F8),
            "wfc1": w18,
            "wfc1r": ((w1 - w18.astype(np.float32)) * RES_K).astype(_F8),
            "wfc2": w28,
            "wfc2r": ((w2 - w28.astype(np.float32)) * RES_K).astype(_F8),
        }
        cfg = self.cfg
        for flag, names in (
            (cfg.apply_ln1_gb, ("ln1_g", "ln1_b")),
            (cfg.apply_ln2_gb, ("ln2_g", "ln2_b")),
            (cfg.apply_qkv_bias, ("qkv_b",)),
            (cfg.apply_proj_bias, ("proj_b",)),
            (cfg.apply_fc1_bias, ("fc1_b",)),
            (cfg.apply_fc2_bias, ("fc2_b",)),
        ):
            if flag:
                for n in names:
                    shared[n] = np.asarray(inputs[n], np.float32)
        per_core = []
        for c in range(_NCORES):
            b, q = divmod(c, _GROUP)
            xc = x[b, q * _T : (q + 1) * _T, :]
            m = {"xT": np.ascontiguousarray(xc.T), **shared}
            per_core.append([m[nm] for nm in self.in_names])
        concat = [np.concatenate([per_core[c][i] for c in range(_NCORES)], axis=0)
                  for i in range(len(self.in_names))]
        concat += [np.zeros((_NCORES * z.shape[0], *z.shape[1:]), z.dtype)
                   for z in self.zero_outs]
        return concat

    def run(self, inputs, n_iters=1):
        args = self.concat_inputs(inputs)
        out = self.fn(n_iters)(*args)
        jax.block_until_ready(out)
        return np.asarray(out[0])

    def measure_ns(self, inputs, n=50, reps=3):
        """Queued-chain wall estimate: upper bound incl. per-dispatch RPC."""
        args = self.concat_inputs(inputs)
        f = self.fn(1)
        lowered = f.lower(*args)
        compiled = lowered.compile()
        shardings = compiled.input_shardings[0]
        dev = [jax.device_put(a, s) for a, s in zip(args, shardings)]
        jax.block_until_ready(dev)
        ix = self.ix_xT
        out = f(*dev)
        jax.block_until_ready(out)
        best = None
        for _ in range(reps):
            x = dev[ix]
            t0 = _time.perf_counter()
            for _i in range(n):
                out = f(*(dev[:ix] + [x] + dev[ix + 1:]))
                x = out[0]
            jax.block_until_ready(out)
            est = (_time.perf_counter() - t0) / n
            best = est if best is None else min(best, est)
        return best * 1e9


def _get_runner(inputs):
    cfg = _build_cfg(inputs)
    key = (cfg.apply_ln1_gb, cfg.apply_ln2_gb, cfg.apply_qkv_bias,
           cfg.apply_proj_bias, cfg.apply_fc1_bias, cfg.apply_fc2_bias)
    if key not in _CACHE:
        _CACHE[key] = _Runner(cfg)
    return _CACHE[key]


def kernel(**inputs) -> np.ndarray:
    r = _get_runner(inputs)
    flat = r.run(inputs)  # [8*DIM, T] stacked per-core outT
    out = np.empty((_B, _N, _DIM), np.float32)
    for c in range(_NCORES):
        b, q = divmod(c, _GROUP)
        out[b, q * _T : (q + 1) * _T, :] = flat[c * _DIM : (c + 1) * _DIM, :].T
    return out


def measure_hw_time_ns(**inputs) -> float:
    """Estimate per-execution device time by differencing chained runs."""
    return _get_runner(inputs).measure_ns(inputs)


# revision 59
# speedup vs baseline: 1.4199x; 1.0195x over previous
"""nn_Block dense_transformer kernel for 8 TRN2 NeuronCores.

Self-contained: builds the Bass/Tile program, shards inputs across 8 cores
(sequence-parallel, 4 cores per batch element), runs via a cached jitted
shard_map over jax's neuron devices, gathers the full output.

Numerics: fp8-e4m3 DoubleRow matmuls for qkv / QK / AV / proj / fc1 (fc1 with
scaled-residual error correction), bf16 fc2, exp->fp8 softmax with a
softmax-invariant score shift; part of the exp work runs on DVE via a
Schraudolph-style uint8 bit trick. Validated ~5.7e-3 rel err vs f32 ref.
"""
import os
import numpy as np
import ml_dtypes


from contextlib import ExitStack
from dataclasses import dataclass

import concourse.bass as bass
import concourse.mybir as mybir
import concourse.tile as tile

F32 = mybir.dt.float32
F32R = mybir.dt.float32r
BF16 = mybir.dt.bfloat16
F8 = mybir.dt.float8e4
U8 = mybir.dt.uint8
P = 128
ALU = mybir.AluOpType
ACTF = mybir.ActivationFunctionType
DR = mybir.MatmulPerfMode.DoubleRow

LOG2E = 1.4426950408889634
RES_K = 32.0   # fc1 residual pre-scale


@dataclass
class Cfg:
    dim: int = 1024
    heads: int = 16
    hd: int = 64
    hidden: int = 4096
    T: int = 512          # tokens per core
    group: int = 4        # cores per batch group
    n_cores: int = 8
    eps: float = 1e-5
    shift: float = 2.6    # softmax-invariant score shift (fit e^s into e4m3)
    # build-time specialization flags (host inspects actual input values)
    apply_ln1_gb: bool = False
    apply_ln2_gb: bool = False
    apply_qkv_bias: bool = False
    apply_proj_bias: bool = False
    apply_fc1_bias: bool = False
    apply_fc2_bias: bool = False
    fake_ag: bool = False  # timing-only: skip collective, read own kv as all ranks
    # j-indices (within each pair's 16 j-steps) whose exp runs on DVE via the
    # Schraudolph uint8 trick; the rest run on ACT (native exp)
    dve_js: tuple = (1, 3, 6, 9, 12, 14)
    schr_round: float = 0.0  # probe-verified: DVE f32->u8 rounds-to-nearest
    stop_after: str = "full"  # bisect: qkv|ag|kvload|attn|proj|fc1|full

    @property
    def dch(self):
        return self.dim // P

    @property
    def hch(self):
        return self.hidden // P

    @property
    def kt(self):
        return (self.group * self.T) // P

    @property
    def tpr(self):
        return self.T // P

    @property
    def pairs(self):
        return self.heads // 2

    @property
    def vf(self):  # free-dim chunk for the v matmul
        return min(512, self.dim)

    @property
    def schr_b(self):
        # uint8(LOG2E * raw_score + schr_b) bitcast e4m3 ~= exp(score/8 - shift)
        return 56.0 - 8.0 * LOG2E * self.shift - 8.0 * 0.043677 + self.schr_round


def build_block(nc: bass.Bass, cfg: Cfg):
    dch, hch, kt, T, hd = cfg.dch, cfg.hch, cfg.kt, cfg.T, cfg.hd

    xT = nc.dram_tensor("xT", [cfg.dim, T], F32, kind="ExternalInput").ap()
    wqkv = nc.dram_tensor("wqkv", [cfg.dim, 3 * cfg.dim], F8, kind="ExternalInput").ap()
    wproj = nc.dram_tensor("wproj", [cfg.dim, cfg.dim], F8, kind="ExternalInput").ap()
    wfc1 = nc.dram_tensor("wfc1", [cfg.dim, cfg.hidden], F8, kind="ExternalInput").ap()
    wfc1r = nc.dram_tensor("wfc1r", [cfg.dim, cfg.hidden], F8, kind="ExternalInput").ap()
    wfc2 = nc.dram_tensor("wfc2", [cfg.hidden, cfg.dim], F8, kind="ExternalInput").ap()
    wfc2r = nc.dram_tensor("wfc2r", [cfg.hidden, cfg.dim], F8, kind="ExternalInput").ap()
    outT = nc.dram_tensor("outT", [cfg.dim, T], F32, kind="ExternalOutput").ap()

    ln1_g = ln1_b = ln2_g = ln2_b = None
    if cfg.apply_ln1_gb:
        ln1_g = nc.dram_tensor("ln1_g", [cfg.dim], F32, kind="ExternalInput").ap()
        ln1_b = nc.dram_tensor("ln1_b", [cfg.dim], F32, kind="ExternalInput").ap()
    if cfg.apply_ln2_gb:
        ln2_g = nc.dram_tensor("ln2_g", [cfg.dim], F32, kind="ExternalInput").ap()
        ln2_b = nc.dram_tensor("ln2_b", [cfg.dim], F32, kind="ExternalInput").ap()
    qkv_b = proj_b = fc1_b = fc2_b = None
    if cfg.apply_qkv_bias:
        qkv_b = nc.dram_tensor("qkv_b", [3 * cfg.dim], F32, kind="ExternalInput").ap()
    if cfg.apply_proj_bias:
        proj_b = nc.dram_tensor("proj_b", [cfg.dim], F32, kind="ExternalInput").ap()
    if cfg.apply_fc1_bias:
        fc1_b = nc.dram_tensor("fc1_b", [cfg.hidden], F32, kind="ExternalInput").ap()
    if cfg.apply_fc2_bias:
        fc2_b = nc.dram_tensor("fc2_b", [cfg.dim], F32, kind="ExternalInput").ap()

    # collective bounce buffers, fp8 (k first so its AllGather can complete and
    # feed QK while v's AllGather still runs)
    half = cfg.dim * T
    cck_in = nc.dram_tensor("cck_in", [half], F8, kind="Internal").ap()
    cck_out = nc.dram_tensor("cck_out", [cfg.group * half], F8, kind="Internal").ap()
    ccv_in = nc.dram_tensor("ccv_in", [half], F8, kind="Internal").ap()
    ccv_out = nc.dram_tensor("ccv_out", [cfg.group * half], F8, kind="Internal").ap()

    def dram_view(ap, off, shape_strides):
        return bass.AP(tensor=ap.tensor, offset=ap.offset + off,
                       ap=[[s, n] for s, n in shape_strides])

    LEVELS = ["qkv", "ag", "kvload", "attn", "proj", "ln2", "fc1", "full"]
    _stop = LEVELS.index(cfg.stop_after)
    go = lambda ph: _stop >= LEVELS.index(ph)  # noqa: E731

    with tile.TileContext(nc) as tc, ExitStack() as ctx:
        const = ctx.enter_context(tc.tile_pool(name="const", bufs=1))
        rows = ctx.enter_context(tc.tile_pool(name="rows", bufs=2))
        bcast = ctx.enter_context(tc.tile_pool(name="bcast", bufs=2))
        evict = ctx.enter_context(tc.tile_pool(name="evict", bufs=3))
        tmp = ctx.enter_context(tc.tile_pool(name="tmp", bufs=2))
        # pool release is strict LIFO: open long-lived pools first
        x2p = ctx.enter_context(tc.tile_pool(name="x2p", bufs=1))
        es_qr = ExitStack()     # qr: lives until end of attention
        es_proj = ExitStack()   # freed after proj: xT, oT, wproj
        es_q = ExitStack()      # qT_8: freed after the qr remap
        wfc1p = ctx.enter_context(tc.tile_pool(name="wfc1", bufs=1))
        qrp = es_qr.enter_context(tc.tile_pool(name="qrp", bufs=1))
        xpool = es_proj.enter_context(tc.tile_pool(name="xpool", bufs=1))
        opool = es_proj.enter_context(tc.tile_pool(name="opool", bufs=1))
        wprojp = es_proj.enter_context(tc.tile_pool(name="wproj", bufs=4))
        qpool = es_q.enter_context(tc.tile_pool(name="qpool", bufs=1))

        ones_bf = const.tile([P, 1], BF16, tag="onesbf", bufs=1)
        nc.vector.memset(ones_bf, 1.0)
        eps_t = const.tile([1, 1], F32, tag="eps", bufs=1)
        nc.vector.memset(eps_t, cfg.eps)
        nshift_t = const.tile([P, 1], F32, tag="nshift", bufs=1)
        nc.vector.memset(nshift_t, -cfg.shift)

        xT_sb = xpool.tile([P, dch, T], F32, name="xT_sb", tag="xT", bufs=1)
        for c in range(0, dch, 2):
            nc.sync.dma_start(
                out=xT_sb[:, c : c + 2, :],
                in_=dram_view(xT, c * P * T, [(T, P), (P * T, 2), (1, T)]))

        def load_cols(vec, nch, nm):
            t = const.tile([P, nch], F32, name=nm, tag=nm, bufs=1)
            nc.sync.dma_start(out=t, in_=vec.rearrange("(c p) -> p c", p=P))
            return t

        ln1g_sb = load_cols(ln1_g, dch, "ln1g") if ln1_g is not None else None
        ln1b_sb = load_cols(ln1_b, dch, "ln1b") if ln1_b is not None else None
        ln2g_sb = load_cols(ln2_g, dch, "ln2g") if ln2_g is not None else None
        ln2b_sb = load_cols(ln2_b, dch, "ln2b") if ln2_b is not None else None
        qkvb_sb = load_cols(qkv_b, 3 * dch, "qkvb") if qkv_b is not None else None
        projb_sb = load_cols(proj_b, dch, "projb") if proj_b is not None else None
        fc1b_sb = load_cols(fc1_b, hch, "fc1b") if fc1_b is not None else None
        fc2b_sb = load_cols(fc2_b, dch, "fc2b") if fc2_b is not None else None

        # ---- LayerNorm stats over features (feature-major data) ----
        # returns (r1b, r2b) broadcast rows: normalized = x*r1b + r2b
        def ln_stats(psln, x_sb):
            sum_x = psln.tile([1, T], F32, name="sum_x", tag="sumx", bufs=1)
            sum_sq = psln.tile([1, T], F32, name="sum_sq", tag="sumsq", bufs=1)
            for c in range(dch):
                eng = nc.gpsimd if c % 2 else nc.vector
                eng2 = nc.vector if c % 2 else nc.gpsimd
                xb = tmp.tile([P, T], BF16, name="xb", tag="xb")
                eng.tensor_copy(out=xb, in_=x_sb[:, c, :])
                xsq = tmp.tile([P, T], BF16, name="xsq", tag="xsq")
                eng2.tensor_mul(xsq, x_sb[:, c, :], x_sb[:, c, :])
                nc.tensor.matmul(sum_x, lhsT=ones_bf, rhs=xb,
                                 start=(c == 0), stop=(c == dch - 1))
                nc.tensor.matmul(sum_sq, lhsT=ones_bf, rhs=xsq,
                                 start=(c == 0), stop=(c == dch - 1))
            inv_d = 1.0 / cfg.dim
            m_row = rows.tile([1, T], F32, name="m_row", bufs=1)
            var_row = rows.tile([1, T], F32, name="var_row", bufs=1)
            nc.vector.tensor_scalar(out=m_row, in0=sum_x, scalar1=inv_d,
                                    scalar2=None, op0=ALU.mult)
            nc.vector.tensor_mul(var_row, m_row, m_row)           # m^2
            nc.vector.scalar_tensor_tensor(                        # sum_sq/D - m^2
                out=var_row, in0=sum_sq, scalar=inv_d, in1=var_row,
                op0=ALU.mult, op1=ALU.subtract)
            lnv = rows.tile([1, T], F32, name="lnv", bufs=1)
            rstd = rows.tile([1, T], F32, name="rstd", bufs=1)
            nc.scalar.activation(lnv, var_row, ACTF.Ln, bias=eps_t)
            nc.scalar.activation(rstd, lnv, ACTF.Exp, scale=-0.5)
            r2 = rows.tile([1, T], F32, name="r2", bufs=1)         # -m * rstd
            nc.vector.scalar_tensor_tensor(
                out=r2, in0=m_row, scalar=-1.0, in1=rstd,
                op0=ALU.mult, op1=ALU.mult)
            r1b = bcast.tile([P, T], F32, name="r1b", bufs=1)
            r2b = bcast.tile([P, T], F32, name="r2b", bufs=1)
            nc.gpsimd.partition_broadcast(r1b, rstd)
            nc.gpsimd.partition_broadcast(r2b, r2)
            return r1b, r2b

        # ---------------- Phase 1: LN1 + QKV(fp8 DR) + AllGather ----------------
        qT_8 = qpool.tile([P, dch, T], F8, name="qT_8")
        with tc.tile_pool(name="h1", bufs=1) as h1p, \
             tc.tile_pool(name="wqkv", bufs=4) as wqkvp, \
             tc.tile_pool(name="psln1", bufs=1, space="PSUM") as psln1, \
             tc.tile_pool(name="psqkv", bufs=4, space="PSUM") as psq:
            h1_8 = h1p.tile([P, dch, T], F8, name="h1_8")
            r1b, r2b = ln_stats(psln1, xT_sb)
            for c in range(dch):
                eng = nc.gpsimd if c % 2 else nc.vector
                t0 = tmp.tile([P, T], F32, name="ln_t0", tag="t0")
                eng.tensor_mul(t0, xT_sb[:, c, :], r1b)
                if ln1g_sb is not None:
                    t1 = tmp.tile([P, T], F32, name="ln_t1", tag="t1")
                    eng.tensor_add(t1, t0, r2b)
                    eng.tensor_scalar(
                        out=h1_8[:, c, :], in0=t1,
                        scalar1=ln1g_sb[:, c : c + 1], scalar2=ln1b_sb[:, c : c + 1],
                        op0=ALU.mult, op1=ALU.add)
                else:
                    eng.tensor_add(h1_8[:, c, :], t0, r2b)

            wq = []
            for u in range(dch // 2):
                w = wqkvp.tile([P, 2, 3 * cfg.dim], F8, name="wqkv_t")
                nc.sync.dma_start(
                    out=w, in_=dram_view(
                        wqkv, 2 * u * P * 3 * cfg.dim,
                        [(3 * cfg.dim, P), (P * 3 * cfg.dim, 2), (1, 3 * cfg.dim)]))
                wq.append(w)

            def qkv_evict(ps, dst, col_idx):
                # on ACT: it is idle during this phase while DVE is loaded
                if qkvb_sb is not None:
                    nc.scalar.activation(dst, ps, ACTF.Identity,
                                         bias=qkvb_sb[:, col_idx : col_idx + 1])
                else:
                    nc.scalar.copy(dst, ps)

            # k first (feeds the AllGather + kr loads), then q (gates the qr
            # remap and so the attention start), then v (only needed at the
            # first AV, a few us into attention). k/q evict on ACT, v on DVE.
            kfull = evict.tile([P, dch, T], F8, name="kfull", tag="kfull", bufs=1)
            for m in range(dch):  # k, feature-major, fp8
                ps = psq.tile([P, T], F32, name="ps_qk", tag="ps_qkv")
                for u in range(dch // 2):
                    nc.tensor.matmul(
                        ps, lhsT=wq[u][:, :, (dch + m) * P : (dch + m + 1) * P],
                        rhs=h1_8[:, 2 * u : 2 * u + 2, :],
                        start=(u == 0), stop=(u == dch // 2 - 1), perf_mode=DR)
                qkv_evict(ps, kfull[:, m, :], dch + m)
                if m % (dch // 2) == dch // 2 - 1:  # store in halves: the
                    m0 = m - dch // 2 + 1           # first kr loads only need
                    nc.sync.dma_start(               # the early feature rows
                        out=dram_view(cck_in, m0 * P * T,
                                      [(T, P), (P * T, dch // 2), (1, T)]),
                        in_=kfull[:, m0 : m + 1, :])
            for m in range(dch):  # q, feature-major (stays local), fp8
                ps = psq.tile([P, T], F32, name="ps_q", tag="ps_qkv")
                for u in range(dch // 2):
                    nc.tensor.matmul(
                        ps, lhsT=wq[u][:, :, m * P : (m + 1) * P],
                        rhs=h1_8[:, 2 * u : 2 * u + 2, :],
                        start=(u == 0), stop=(u == dch // 2 - 1), perf_mode=DR)
                qkv_evict(ps, qT_8[:, m, :], m)
            vfull = evict.tile([P, cfg.tpr, 2, cfg.vf], F8, name="vfull",
                               tag="vfull", bufs=1)
            for tm in range(cfg.tpr):  # v, token-major, fp8
                for f in range(cfg.dim // cfg.vf):
                    ps = psq.tile([P, cfg.vf], F32, name="ps_v", tag="ps_qkv")
                    for u in range(dch // 2):
                        nc.tensor.matmul(
                            ps, lhsT=h1_8[:, 2 * u : 2 * u + 2, tm * P : (tm + 1) * P],
                            rhs=wq[u][:, :, 2 * cfg.dim + f * cfg.vf
                                      : 2 * cfg.dim + (f + 1) * cfg.vf],
                            start=(u == 0), stop=(u == dch // 2 - 1), perf_mode=DR)
                    # v bias folded in after the softmax (rows sum to 1)
                    nc.vector.tensor_copy(out=vfull[:, tm, f, :], in_=ps)
            nc.sync.dma_start(
                out=dram_view(ccv_in, 0,
                              [(cfg.dim, P), (P * cfg.dim, cfg.tpr), (1, cfg.dim)]),
                in_=vfull)

        if go("ag") and not cfg.fake_ag:
            groups = [list(range(g * cfg.group, (g + 1) * cfg.group))
                      for g in range(cfg.n_cores // cfg.group)]
            nc.gpsimd.collective_compute(
                "AllGather", ALU.bypass,
                ins=[cck_in], outs=[cck_out], replica_groups=groups)
            nc.gpsimd.collective_compute(
                "AllGather", ALU.bypass,
                ins=[ccv_in], outs=[ccv_out], replica_groups=groups)

        # proj + fc1 weight tiles (fp8, c-pair layout for DoubleRow); allocated
        # now, but DMA issue is deferred into the attention loop so the
        # attention-critical kr/v loads hit the DMA path first
        wp = [wprojp.tile([P, 2, cfg.dim], F8, name="wproj_t")
              for _ in range(dch // 2)]
        w1m = [wfc1p.tile([P, 2, cfg.hidden], F8, name="wfc1_m", tag=f"w1m{u}",
                          bufs=1) for u in range(dch // 2)]
        w1r = [wfc1p.tile([P, 2, cfg.hidden], F8, name="wfc1_r", tag=f"w1r{u}",
                          bufs=1) for u in range(dch // 2)]

        def pair_view(nrow):
            return [(nrow, P), (P * nrow, 2), (1, nrow)]

        _deferred = []
        for u in range(dch // 2):
            _deferred.append((wp[u], dram_view(
                wproj, 2 * u * P * cfg.dim, pair_view(cfg.dim))))
        for u in range(dch // 2):
            _deferred.append((w1m[u], dram_view(
                wfc1, 2 * u * P * cfg.hidden, pair_view(cfg.hidden))))
            _deferred.append((w1r[u], dram_view(
                wfc1r, 2 * u * P * cfg.hidden, pair_view(cfg.hidden))))

        # ---------------- Phase 2: attention ----------------
        # q remap to DoubleRow layout [32, half, head, T] via 4 SBUF DMAs
        qr = qrp.tile([32, 2, cfg.heads, T], F8, name="qr")
        for hp in range(2):
            for hf in range(2):
                p0 = hp * 64 + hf * 32
                nc.sync.dma_start(
                    out=qr[:, hf, hp : cfg.heads : 2, :],
                    in_=qT_8[p0 : p0 + 32, :, :])
        es_q.close()  # qT_8 freed

        oT_8 = opool.tile([P, dch, T], F8, name="oT_8")
        with tc.tile_pool(name="krp", bufs=2) as krp, \
             tc.tile_pool(name="vaugp", bufs=1) as vaugp, \
             tc.tile_pool(name="expp", bufs=3) as expp, \
             tc.tile_pool(name="psS", bufs=3, space="PSUM") as psS, \
             tc.tile_pool(name="psO", bufs=1, space="PSUM") as psO:
            def load_kr(p):
                # k for pair p, DoubleRow layout [32, head(2), half, keys];
                # (head, half) merge on both sides -> one DMA per rank
                kr = krp.tile([32, 2, 2, cfg.group * T], F8, name="kr", tag="kr")
                for r in range(cfg.group):
                    ksrc = [(T, 32), (32 * T, 4), (1, T)]
                    koff = 2 * p * hd * T
                    nc.sync.dma_start(
                        out=kr[:, :, :, r * T : (r + 1) * T],
                        in_=dram_view(cck_in, koff, ksrc) if cfg.fake_ag
                        else dram_view(cck_out, r * half + koff, ksrc))
                return kr

            v_aug = vaugp.tile([P, kt, cfg.heads, hd + 1], F8, name="v_aug")
            nc.vector.memset(v_aug[:, :, :, hd : hd + 1], 1.0)
            kr_next = load_kr(0) if go("attn") else None  # before the v stream
            for r in range(cfg.group if go("kvload") else 0):
                for jt in range(cfg.tpr):
                    vsrc = [(cfg.dim, P), (hd, cfg.heads), (1, hd)]
                    voff = jt * P * cfg.dim
                    nc.sync.dma_start(
                        out=v_aug[:, r * cfg.tpr + jt, :, :hd],
                        in_=dram_view(ccv_in, voff, vsrc) if cfg.fake_ag
                        else dram_view(ccv_out, r * half + voff, vsrc))

            x2_sb = x2p.tile([P, dch, T], F32, name="x2_sb", tag="x2", bufs=1)
            for p in range(cfg.pairs if go("attn") else 0):
                kr = kr_next
                if p + 1 < cfg.pairs:
                    kr_next = load_kr(p + 1)
                # drip the deferred proj/fc1 weight loads into the late
                # (DMA-idle) half of the attention window
                if p >= 4:
                    for w, src in _deferred[4 * (p - 4) : 4 * (p - 3)]:
                        nc.sync.dma_start(out=w, in_=src)

                psO_A = psO.tile([hd + 1, T], F32, name="psO_A", tag="oA")
                psO_B = psO.tile([hd + 1, T], F32, name="psO_B", tag="oB")
                for uj in range(kt // 2):
                    es = expp.tile([P, 2, 2, T], F8, name="es", tag="es")
                    es_u8 = es.bitcast(U8)
                    for i in range(2):
                        j = 2 * uj + i
                        s = psS.tile([P, 2, T], F32, name="s_qk", tag="s")
                        kcol = slice(j * P, (j + 1) * P)
                        for hh in range(2):
                            nc.tensor.matmul(
                                s[:, hh, :], lhsT=kr[:, hh, :, kcol],
                                rhs=qr[:, :, 2 * p + hh, :],
                                start=True, stop=True, perf_mode=DR)
                        if j in cfg.dve_js:
                            # Schraudolph exp on DVE: uint8(A*raw+B) bits = e4m3
                            nc.vector.tensor_scalar(
                                out=es_u8[:, i, :, :], in0=s,
                                scalar1=LOG2E, scalar2=cfg.schr_b,
                                op0=ALU.mult, op1=ALU.add)
                        else:
                            nc.scalar.activation(
                                es[:, i, :, :], s, ACTF.Exp,
                                scale=0.125, bias=nshift_t)
                    nc.tensor.matmul(
                        psO_A, lhsT=v_aug[:, 2 * uj : 2 * uj + 2, 2 * p, :],
                        rhs=es[:, :, 0, :],
                        start=(uj == 0), stop=(uj == kt // 2 - 1), perf_mode=DR)
                    nc.tensor.matmul(
                        psO_B, lhsT=v_aug[:, 2 * uj : 2 * uj + 2, 2 * p + 1, :],
                        rhs=es[:, :, 1, :],
                        start=(uj == 0), stop=(uj == kt // 2 - 1), perf_mode=DR)
                for h, pso, poff in ((2 * p, psO_A, 0), (2 * p + 1, psO_B, 64)):
                    rrow = rows.tile([1, T], F32, name="rrow")
                    nc.vector.reciprocal(rrow, pso[hd : hd + 1, :])
                    rb = bcast.tile([P, T], F32, name="rb")
                    nc.gpsimd.partition_broadcast(rb[0:hd, :], rrow)
                    dst = oT_8[poff : poff + hd, p, :]
                    nc.vector.tensor_mul(dst, pso[0:hd, :], rb[0:hd, :])
                    if qkvb_sb is not None:  # v bias: softmax rows sum to 1
                        nc.vector.tensor_scalar_add(
                            out=dst, in0=dst,
                            scalar1=qkvb_sb[poff : poff + hd,
                                            2 * dch + p : 2 * dch + p + 1])

        # ---------------- proj (fp8 DR, PSUM-accumulated) + residual ----------
        if go("proj"):
            with tc.tile_pool(name="pspp", bufs=2, space="PSUM") as pspp:
                for m in range(dch):
                    pp = pspp.tile([P, T], F32, name="ps_pp")
                    for u in range(dch // 2):
                        nc.tensor.matmul(
                            pp, lhsT=wp[u][:, :, m * P : (m + 1) * P],
                            rhs=oT_8[:, 2 * u : 2 * u + 2, :],
                            start=(u == 0), stop=(u == dch // 2 - 1), perf_mode=DR)
                    if projb_sb is not None:
                        nc.vector.tensor_scalar_add(
                            out=pp, in0=pp, scalar1=projb_sb[:, m : m + 1])
                    nc.vector.tensor_add(x2_sb[:, m, :], pp, xT_sb[:, m, :])

        es_proj.close()  # xT, oT, wproj no longer needed
        es_qr.close()    # qr freed

        # ------------- Phase 3: LN2 + fc1(fp8 DR, corrected) + gelu + fc2 -----
        gpool = ctx.enter_context(tc.tile_pool(name="gpool", bufs=1))
        g8_sb = gpool.tile([P, hch, T], F8, name="g8_sb")
        g8d_sb = gpool.tile([P, hch, T], F8, name="g8d_sb")
        l28_sb = gpool.tile([P, hch, T], F8, name="l28_sb")
        with tc.tile_pool(name="h2", bufs=1) as h2p:
            h2_8 = h2p.tile([P, dch, T], F8, name="h2_8")
            h2d_8 = h2p.tile([P, dch, T], F8, name="h2d_8")
            l2_8 = h2p.tile([P, dch, T], F8, name="l2_8")
            with tc.tile_pool(name="psln2", bufs=1, space="PSUM") as psln2:
                if go("ln2"):
                    r1b, r2b = ln_stats(psln2, x2_sb)
                    for c in range(dch):
                        eng = nc.gpsimd if c % 2 else nc.vector
                        t0 = tmp.tile([P, T], F32, name="l2t0", tag="t0")
                        t1 = tmp.tile([P, T], F32, name="l2t1", tag="t1")
                        eng.tensor_mul(t0, x2_sb[:, c, :], r1b)
                        if ln2g_sb is not None:
                            eng.tensor_add(t1, t0, r2b)
                            eng.tensor_scalar(
                                out=t1, in0=t1,
                                scalar1=ln2g_sb[:, c : c + 1],
                                scalar2=ln2b_sb[:, c : c + 1],
                                op0=ALU.mult, op1=ALU.add)
                        else:
                            eng.tensor_add(t1, t0, r2b)
                        # fc1 correction operands, pre-scaled so all three
                        # matmul series accumulate into ONE psum bank:
                        #   h8*W8 + fp8(t1/K)*RW8 + fp8(t1-h8)*W8
                        nc.scalar.copy(h2_8[:, c, :], t1)
                        nc.scalar.mul(h2d_8[:, c, :], t1, 1.0 / RES_K)
                        nc.vector.tensor_tensor(
                            out=l2_8[:, c, :], in0=t1, in1=h2_8[:, c, :],
                            op=ALU.subtract)

            with tc.tile_pool(name="psm", bufs=4, space="PSUM") as psm:
                for m in range(hch if go("fc1") else 0):
                    ps = psm.tile([P, T], F32, name="ps_fc1", tag="fm")
                    for u in range(dch // 2):
                        nc.tensor.matmul(
                            ps, lhsT=w1m[u][:, :, m * P : (m + 1) * P],
                            rhs=h2_8[:, 2 * u : 2 * u + 2, :],
                            start=(u == 0), stop=False, perf_mode=DR)
                    for u in range(dch // 2):
                        nc.tensor.matmul(
                            ps, lhsT=w1r[u][:, :, m * P : (m + 1) * P],
                            rhs=h2d_8[:, 2 * u : 2 * u + 2, :],
                            start=False, stop=False, perf_mode=DR)
                    for u in range(dch // 2):
                        nc.tensor.matmul(
                            ps, lhsT=w1m[u][:, :, m * P : (m + 1) * P],
                            rhs=l2_8[:, 2 * u : 2 * u + 2, :],
                            start=False, stop=(u == dch // 2 - 1), perf_mode=DR)
                    # gelu in bf16, then fp8 main + W/X correction operands
                    # (same single-psum-accumulation trick as fc1)
                    gbf = tmp.tile([P, T], BF16, name="gbf", tag="gbf")
                    nc.scalar.activation(
                        gbf, ps, ACTF.Gelu,
                        bias=fc1b_sb[:, m : m + 1] if fc1b_sb is not None else 0.0)
                    nc.gpsimd.tensor_copy(out=g8_sb[:, m, :], in_=gbf)
                    nc.vector.tensor_tensor(
                        out=l28_sb[:, m, :], in0=gbf, in1=g8_sb[:, m, :],
                        op=ALU.subtract)
                    eng = nc.gpsimd if m % 2 else nc.vector
                    eng.tensor_scalar(
                        out=g8d_sb[:, m, :], in0=gbf, scalar1=1.0 / RES_K,
                        scalar2=None, op0=ALU.mult)

        with tc.tile_pool(name="wfc2", bufs=4) as wfc2p, \
             tc.tile_pool(name="psf2", bufs=1, space="PSUM") as psf2:
            acc = [psf2.tile([P, T], F32, name=f"ps_fc2_{m}", tag=f"acc{m}",
                             bufs=1) for m in range(dch)]
            ncu = hch // 2
            for cu in range(ncu if go("full") else 0):
                w2 = wfc2p.tile([P, 2, cfg.dim], F8, name="wfc2_t", tag="w2m")
                w2r = wfc2p.tile([P, 2, cfg.dim], F8, name="wfc2r_t", tag="w2r")
                nc.sync.dma_start(out=w2, in_=dram_view(
                    wfc2, 2 * cu * P * cfg.dim, pair_view(cfg.dim)))
                nc.sync.dma_start(out=w2r, in_=dram_view(
                    wfc2r, 2 * cu * P * cfg.dim, pair_view(cfg.dim)))
                for m in range(dch):
                    lhs = slice(m * P, (m + 1) * P)
                    cs = slice(2 * cu, 2 * cu + 2)
                    nc.tensor.matmul(
                        acc[m], lhsT=w2[:, :, lhs], rhs=g8_sb[:, cs, :],
                        start=(cu == 0), stop=False, perf_mode=DR)
                    nc.tensor.matmul(
                        acc[m], lhsT=w2r[:, :, lhs], rhs=g8d_sb[:, cs, :],
                        start=False, stop=False, perf_mode=DR)
                    nc.tensor.matmul(
                        acc[m], lhsT=w2[:, :, lhs], rhs=l28_sb[:, cs, :],
                        start=False, stop=(cu == ncu - 1), perf_mode=DR)
            # final residual add in place into x2, then store in two DMAs
            for m in range(dch if go("full") else 0):
                if fc2b_sb is not None:
                    nc.vector.tensor_scalar_add(
                        out=acc[m], in0=acc[m], scalar1=fc2b_sb[:, m : m + 1])
                nc.vector.tensor_add(x2_sb[:, m, :], acc[m], x2_sb[:, m, :])
                if go("full") and m % (dch // 2) == dch // 2 - 1:
                    m0 = m - dch // 2 + 1
                    nc.sync.dma_start(
                        out=dram_view(outT, m0 * P * T,
                                      [(T, P), (P * T, dch // 2), (1, T)]),
                        in_=x2_sb[:, m0 : m + 1, :])

    return nc


# ----------------------------------------------------------------------------
# host wrapper
# ----------------------------------------------------------------------------
import time as _time

import jax
from jax.sharding import Mesh, PartitionSpec
from jax.experimental.shard_map import shard_map

from concourse import bacc
from concourse.bass2jax import (_bass_exec_p, install_neuronx_cc_hook,
                                partition_id_tensor)

_BF = ml_dtypes.bfloat16
_F8 = ml_dtypes.float8_e4m3
_DIM, _HEADS, _HIDDEN = 1024, 16, 4096
_B, _N = 2, 2048
_GROUP, _NCORES = 4, 8
_T = _B * _N // _NCORES

_CACHE = {}


def _build_cfg(inputs):
    def nz(a):
        return bool(np.any(np.asarray(a)))

    return Cfg(
        dim=_DIM, heads=_HEADS, hidden=_HIDDEN, T=_T, group=_GROUP,
        n_cores=_NCORES,
        apply_ln1_gb=not (np.allclose(inputs["ln1_g"], 1.0)
                          and not nz(inputs["ln1_b"])),
        apply_ln2_gb=not (np.allclose(inputs["ln2_g"], 1.0)
                          and not nz(inputs["ln2_b"])),
        apply_qkv_bias=nz(inputs["qkv_b"]),
        apply_proj_bias=nz(inputs["proj_b"]),
        apply_fc1_bias=nz(inputs["fc1_b"]),
        apply_fc2_bias=nz(inputs["fc2_b"]),
    )


def _prefer_act_tables(arch):
    """No-op: reordering the cached act-table dict desyncs act_func_set_id
    (a positional index into act_info.json) from the walrus-side mapping."""


class _Runner:
    def __init__(self, cfg):
        import concourse.mybir as mybir

        self.cfg = cfg
        nc = bacc.Bacc("TRN2", target_bir_lowering=False, debug=False,
                       num_devices=_NCORES)
        build_block(nc, cfg)
        _prefer_act_tables(nc.m.arch)
        nc.compile()
        self.nc = nc
        install_neuronx_cc_hook()

        in_names, out_names, out_avals, zero_outs = [], [], [], []
        pid = nc.partition_id_tensor.name if nc.partition_id_tensor else None
        self.pid_name = pid
        for alloc in nc.m.functions[0].allocations:
            if not isinstance(alloc, mybir.MemoryLocationSet):
                continue
            name = alloc.memorylocations[0].name
            if alloc.kind == "ExternalInput":
                if name != pid:
                    in_names.append(name)
            elif alloc.kind == "ExternalOutput":
                out_names.append(name)
                shape = tuple(alloc.tensor_shape)
                dtype = mybir.dt.np(alloc.dtype)
                out_avals.append(jax.core.ShapedArray(shape, dtype))
                zero_outs.append(np.zeros(shape, dtype))
        assert out_names == ["outT"]
        self.in_names = in_names
        self.out_names = out_names
        self.out_avals = out_avals
        self.zero_outs = zero_outs
        self.ix_xT = in_names.index("xT")
        self.fns = {}

    def fn(self, n_iters):
        if n_iters in self.fns:
            return self.fns[n_iters]
        n_params = len(self.in_names)
        all_in = tuple(self.in_names + self.out_names
                       + ([self.pid_name] if self.pid_name else []))
        pid = self.pid_name
        out_avals = tuple(self.out_avals)
        out_names = tuple(self.out_names)
        nc = self.nc
        ix = self.ix_xT

        def _call(x, ins, zouts):
            operands = ins[:ix] + [x] + ins[ix + 1:] + zouts
            if pid:
                operands = operands + [partition_id_tensor()]
            outs = _bass_exec_p.bind(
                *operands, out_avals=out_avals, in_names=all_in,
                out_names=out_names, lowering_input_output_aliases=(),
                sim_require_finite=True, sim_require_nnan=True, nc=nc)
            return outs[0]

        def _body(*args):
            ins = list(args[:n_params])
            zouts = list(args[n_params:])
            x = ins[ix]
            if n_iters == 1:
                return (_call(x, ins, zouts),)
            # neuronx_cc_hook allows one bass_exec per XLA module: use scan
            import jax.lax as lax

            def step(carry, _):
                return _call(carry, ins, zouts), None

            x, _ = lax.scan(step, x, None, length=n_iters)
            return (x,)

        mesh = Mesh(np.asarray(jax.devices()[:_NCORES]), ("core",))
        specs = (PartitionSpec("core"),) * (n_params + 1)
        f = jax.jit(shard_map(_body, mesh=mesh, in_specs=specs,
                              out_specs=(PartitionSpec("core"),),
                              check_rep=False))
        self.fns[n_iters] = f
        return f

    def concat_inputs(self, inputs):
        x = np.asarray(inputs["x"], np.float32)
        w1 = np.asarray(inputs["fc1_w"], np.float32)
        w18 = w1.astype(_F8)
        w2 = np.asarray(inputs["fc2_w"], np.float32)
        w28 = w2.astype(_F8)
        shared = {
            "wqkv": np.ascontiguousarray(
                np.asarray(inputs["qkv_w"], np.float32)).astype(_F8),
            "wproj": np.asarray(inputs["proj_w"], np.float32).astype(_F8),
            "wfc1": w18,
            "wfc1r": ((w1 - w18.astype(np.float32)) * RES_K).astype(_F8),
            "wfc2": w28,
            "wfc2r": ((w2 - w28.astype(np.float32)) * RES_K).astype(_F8),
        }
        cfg = self.cfg
        for flag, names in (
            (cfg.apply_ln1_gb, ("ln1_g", "ln1_b")),
            (cfg.apply_ln2_gb, ("ln2_g", "ln2_b")),
            (cfg.apply_qkv_bias, ("qkv_b",)),
            (cfg.apply_proj_bias, ("proj_b",)),
            (cfg.apply_fc1_bias, ("fc1_b",)),
            (cfg.apply_fc2_bias, ("fc2_b",)),
        ):
            if flag:
                for n in names:
                    shared[n] = np.asarray(inputs[n], np.float32)
        per_core = []
        for c in range(_NCORES):
            b, q = divmod(c, _GROUP)
            xc = x[b, q * _T : (q + 1) * _T, :]
            m = {"xT": np.ascontiguousarray(xc.T), **shared}
            per_core.append([m[nm] for nm in self.in_names])
        concat = [np.concatenate([per_core[c][i] for c in range(_NCORES)], axis=0)
                  for i in range(len(self.in_names))]
        concat += [np.zeros((_NCORES * z.shape[0], *z.shape[1:]), z.dtype)
                   for z in self.zero_outs]
        return concat

    def run(self, inputs, n_iters=1):
        args = self.concat_inputs(inputs)
        out = self.fn(n_iters)(*args)
        jax.block_until_ready(out)
        return np.asarray(out[0])

    def measure_ns(self, inputs, n=50, reps=3):
        """Queued-chain wall estimate: upper bound incl. per-dispatch RPC."""
        args = self.concat_inputs(inputs)
        f = self.fn(1)
        lowered = f.lower(*args)
        compiled = lowered.compile()
        shardings = compiled.input_shardings[0]
        dev = [jax.device_put(a, s) for a, s in zip(args, shardings)]
        jax.block_until_ready(dev)
        ix = self.ix_xT
        out = f(*dev)
        jax.block_until_ready(out)
        best = None
        for _ in range(reps):
            x = dev[ix]
            t0 = _time.perf_counter()
            for _i in range(n):
                out = f(*(dev[:ix] + [x] + dev[ix + 1:]))
                x = out[0]
            jax.block_until_ready(out)
            est = (_time.perf_counter() - t0) / n
            best = est if best is None else min(best, est)
        return best * 1e9


def _get_runner(inputs):
    cfg = _build_cfg(inputs)
    key = (cfg.apply_ln1_gb, cfg.apply_ln2_gb, cfg.apply_qkv_bias,
           cfg.apply_proj_bias, cfg.apply_fc1_bias, cfg.apply_fc2_bias)
    if key not in _CACHE:
        _CACHE[key] = _Runner(cfg)
    return _CACHE[key]


def kernel(**inputs) -> np.ndarray:
    r = _get_runner(inputs)
    flat = r.run(inputs)  # [8*DIM, T] stacked per-core outT
    out = np.empty((_B, _N, _DIM), np.float32)
    for c in range(_NCORES):
        b, q = divmod(c, _GROUP)
        out[b, q * _T : (q + 1) * _T, :] = flat[c * _DIM : (c + 1) * _DIM, :].T
    return out


def measure_hw_time_ns(**inputs) -> float:
    """Estimate per-execution device time by differencing chained runs."""
    return _get_runner(inputs).measure_ns(inputs)
